# revision 29
# baseline (speedup 1.0000x reference)
"""GCNHead Trainium2 kernel (8-core SPMD), v2.

Math (matches reference):
  deg = bincount(dst)+1 (self loops);  dinv = deg^-1/2
  agg[n] = sum_{e: dst=n} dinv[src]*dinv[dst] * x[src]   (+ self loop)
  h = agg[n] @ W_gcn + b_gcn
  out = leaky_relu(h, 0.2);  pooled = segment_max(out, batch)
  result = pooled @ (W_fc / sigma_max(W_fc)).T + b_fc

v2 design: the edge norm dinv[src]*dinv[dst] is folded into a VALUED one-hot
(entry = norm instead of 1), so the x~ prescale pass of v1 disappears and the
gather reads raw bf16 x directly (saves a full read+write of x and removes the
serial prefix before gathering can start).  The valued one-hot is built on DVE
(is_equal vs iota, then mult by the norm table; GNN_NDVE batches) with a
gpsimd local_scatter fallback for the rest -- NDVE=14 (all DVE) measured
fastest since Pool is saturated by gather descriptor generation.  The
spectral-norm sigma(W_fc) chain is hoisted out of the repeat loop (it is a
~40-op serial dependency chain that otherwise adds ~100us of semaphore
latency after pooling); bias+leaky_relu is fused into one Activation-engine
Lrelu op.  Device-verified limits: dma_gather calls are capped at 1024
descriptors; SBUF-source dma_gather is not supported by this runtime.

Host does integer-only preprocessing (sharding, bucketing, index tables,
degree products via bincount); all float math runs on device.
"""
import sys

sys.path.insert(0, "/opt/trn_rl_repo")

import math
import os
import numpy as np
import ml_dtypes

import concourse.bass as bass
import concourse.mybir as mybir
import concourse.tile as tile
from concourse import bacc
from concourse.masks import make_identity

BF16 = mybir.dt.bfloat16
F32 = mybir.dt.float32
I16 = mybir.dt.int16

NCORES = 8
SENT = 20000.0  # dst_local sentinel (never matches iota 0..63)
BB = 8          # tiles per bank-batch
CH = 30         # blocks per local_scatter call (30*64=1920 < 2048 elems)


# ----------------------------------------------------------------------------
# Host preprocessing (integers only)
# ----------------------------------------------------------------------------
def _preprocess(x, edge_index, batch, num_graphs):
    N, D = x.shape
    B = int(num_graphs)
    src = np.asarray(edge_index[0], dtype=np.int64)
    dst = np.asarray(edge_index[1], dtype=np.int64)
    batch = np.asarray(batch, dtype=np.int64)

    deg = np.bincount(dst, minlength=N).astype(np.int64) + 1  # + self loop

    # graph -> node range (batch sorted)
    counts_g = np.bincount(batch, minlength=B)
    starts_g = np.concatenate([[0], np.cumsum(counts_g)])

    GPC = math.ceil(B / NCORES)  # graphs per core
    # balance graphs across cores by total edge weight (snake over sorted)
    gw = np.add.reduceat(deg, starts_g[:-1]) if N else counts_g
    gw = np.where(counts_g > 0, gw, 0)
    order = np.argsort(-gw, kind="stable")
    core_graphs = [[] for _ in range(NCORES)]
    loads = np.zeros(NCORES)
    for g in order:
        c = int(np.argmin([loads[i] + (1e18 if len(core_graphs[i]) >= GPC else 0)
                           for i in range(NCORES)]))
        core_graphs[c].append(int(g))
        loads[c] += gw[g]
    for c in range(NCORES):
        core_graphs[c] += [-1] * (GPC - len(core_graphs[c]))

    GCAP = 64 * max(1, math.ceil(counts_g.max() / 64))
    TPG = GCAP // 64          # tiles per graph
    TT = GPC * TPG            # tiles per core
    S = TT * 64               # slots per core

    # --- slot assignment: per graph, balance node degree across TPG bins ---
    node_slot = np.full(N, -1, dtype=np.int64)   # slot within its core
    node_core = np.full(N, -1, dtype=np.int64)
    for c in range(NCORES):
        for gi, g in enumerate(core_graphs[c]):
            if g < 0:
                continue
            nodes = np.arange(starts_g[g], starts_g[g + 1])
            if len(nodes) == 0:
                continue
            nd = deg[nodes]
            ordn = np.argsort(-nd, kind="stable")
            binload = np.zeros(TPG, dtype=np.int64)
            binfill = np.zeros(TPG, dtype=np.int64)
            for i in ordn:
                masked = np.where(binfill < 64, binload, np.iinfo(np.int64).max)
                b = int(np.argmin(masked))
                slot = gi * GCAP + b * 64 + binfill[b]
                node_slot[nodes[i]] = slot
                node_core[nodes[i]] = c
                binfill[b] += 1
                binload[b] += nd[i]

    # --- edges (incl self loops) bucketed per (core, tile, parity) ---
    loop = np.arange(N, dtype=np.int64)
    esrc = np.concatenate([src, loop])
    edst = np.concatenate([dst, loop])
    ecore = node_core[edst]
    eslot = node_slot[edst]
    etile = eslot >> 6
    edl = (eslot & 63).astype(np.int64)
    epar = (esrc & 1).astype(np.int64)

    # per-core lists of (tile, parity) buckets; order edges by key
    counts = np.zeros((NCORES, TT, 2), dtype=np.int64)
    per_core_order = []
    for c in range(NCORES):
        sel = np.where(ecore == c)[0]
        k = etile[sel] * 2 + epar[sel]
        o = np.argsort(k, kind="stable")
        sel = sel[o]
        per_core_order.append(sel)
        cnt = np.bincount(k[o], minlength=TT * 2)
        counts[c] = cnt.reshape(TT, 2)

    # class capacity = max over cores, rounded to 128
    cap = ((counts.max(axis=0) + 127) // 128) * 128      # [TT, 2]
    blocks = cap // 128                                   # [TT, 2]
    TOTBLK = int(blocks.sum())
    TOTPOS = TOTBLK * 128

    # global block layout: tiles ascending, class even then odd
    class_off = np.zeros((TT, 2), dtype=np.int64)        # position offsets
    pos = 0
    for t in range(TT):
        for q in range(2):
            class_off[t, q] = pos
            pos += cap[t, q]

    # per-tile first block; bank-batch block spans (even-rounded for scatter)
    tile_blk0 = np.zeros(TT + 1, dtype=np.int64)
    for t in range(TT):
        tile_blk0[t + 1] = tile_blk0[t] + blocks[t, 0] + blocks[t, 1]
    nbatch = math.ceil(TT / BB)
    batch_blk0 = np.array([tile_blk0[min(b * BB, TT)] for b in range(nbatch + 1)])
    batch_nblk = np.diff(batch_blk0)
    batch_nblk_e = (batch_nblk + 1) // 2 * 2              # even-rounded
    ebl0 = np.concatenate([[0], np.cumsum(batch_nblk_e)])
    TOTBLKE = int(ebl0[-1])

    # tables
    idx_tab = np.zeros((NCORES, TOTPOS), dtype=np.int64)
    dstl_tab = np.full((NCORES, 128, TOTBLKE), SENT, dtype=np.float32)
    sidx_tab = np.full((NCORES, 128, TOTBLKE), -1, dtype=np.int16)
    degp_tab = np.ones((NCORES, 128, TOTBLKE), dtype=np.float32)
    blk_batch = np.searchsorted(batch_blk0[1:], np.arange(TOTBLK), side="right")
    blk_ecol = ebl0[blk_batch] + (np.arange(TOTBLK) - batch_blk0[blk_batch])
    blk_local = np.arange(TOTBLK) - batch_blk0[blk_batch]
    for c in range(NCORES):
        sel = per_core_order[c]
        k = etile[sel] * 2 + epar[sel]
        # position within class = running index per class
        cstart = np.concatenate([[0], np.cumsum(np.bincount(k, minlength=TT * 2))])
        within = np.arange(len(sel)) - cstart[k]
        gpos = class_off.reshape(-1)[k] + within
        idx_tab[c, gpos] = esrc[sel] >> 1
        blk = gpos >> 7
        lane = gpos & 127
        ecol = blk_ecol[blk]
        dstl_tab[c, lane, ecol] = edl[sel]
        sidx_tab[c, lane, ecol] = (blk_local[blk] % CH) * 64 + edl[sel]
        degp_tab[c, lane, ecol] = deg[esrc[sel]] * deg[edst[sel]]

    # idx table SBUF layout [128, TOTPOS/16]: flat i -> [i%16 (+16r), i//16]
    idx16 = idx_tab.astype(np.int16).reshape(NCORES, TOTPOS // 16, 16)
    idx16 = np.ascontiguousarray(idx16.transpose(0, 2, 1))           # [NC,16,P/16]
    idx128 = np.tile(idx16, (1, 8, 1))                               # [NC,128,...]
    dstl128 = dstl_tab.astype(ml_dtypes.bfloat16)

    NP = ((N + 255) // 256) * 256          # pad to even multiple of 128
    x_pad = np.zeros((NP, D), dtype=ml_dtypes.bfloat16)
    x_pad[:N] = np.asarray(x, dtype=np.float32).astype(ml_dtypes.bfloat16)
    # SBUF-resident pair layout: pair p -> partition p%128, rank p//128
    NPAIR = NP // 2
    RNK = NPAIR // 128
    x_sb = np.ascontiguousarray(
        x_pad.reshape(RNK, 128, 2 * D).transpose(1, 0, 2)
    ).reshape(128, RNK * 2 * D)

    dims = dict(N=N, D=D, B=B, GPC=GPC, GCAP=GCAP, TPG=TPG, TT=TT, S=S, NP=NP,
                TOTBLK=TOTBLK, TOTPOS=TOTPOS, TOTBLKE=TOTBLKE,
                blocks=tuple(map(tuple, blocks)),
                kmax=tuple(map(tuple, counts.max(axis=0))),
                batch_nblk_e=tuple(batch_nblk_e), ebl0=tuple(ebl0))
    tables = dict(idx=idx128, dstl=dstl128, sidx=sidx_tab, degp=degp_tab,
                  x_pad=x_pad, x_sb=x_sb, core_graphs=core_graphs)
    return dims, tables


# ----------------------------------------------------------------------------
# Device program
# ----------------------------------------------------------------------------
def _build_program(dims):
    D = dims["D"]
    TT, TPG, GPC, GCAP = dims["TT"], dims["TPG"], dims["GPC"], dims["GCAP"]
    NP, TOTBLK, TOTPOS = dims["NP"], dims["TOTBLK"], dims["TOTPOS"]
    TOTBLKE = dims["TOTBLKE"]
    blocks = dims["blocks"]
    kmax = dims["kmax"]
    batch_nblk_e = dims["batch_nblk_e"]
    ebl0 = dims["ebl0"]
    S = dims["S"]

    SCRATCH = int(os.environ.get("GNN_SCRATCH", "16384"))
    GSTEP = int(os.environ.get("GNN_GSTEP", "8"))         # blocks per gather
    # NOTE: the device rejects gather calls over 1024 descriptors (GSTEP>8).
    NDVE = int(os.environ.get("GNN_NDVE", "14"))          # batches on DVE path
    NQ = int(os.environ.get("GNN_NQ", "4"))
    LRELU = int(os.environ.get("GNN_LRELU", "1"))
    GSRC = os.environ.get("GNN_GSRC", "hbm")              # hbm | sbuf
    SPKT = bool(int(os.environ.get("GNN_SPKT", "1")))

    nc = bacc.Bacc("TRN2", target_bir_lowering=False, debug=False,
                   num_swdge_queues=NQ, dynamic_dma_scratch_size=SCRATCH)
    if GSRC == "sbuf":
        RNK = (NP // 2) // 128
        xsb_d = nc.dram_tensor("xsb", [128, RNK * 2 * D], BF16,
                               kind="ExternalInput")
    else:
        x_d = nc.dram_tensor("x", [NP, D], BF16, kind="ExternalInput")
    idx_d = nc.dram_tensor("idx", [128, TOTPOS // 16], I16, kind="ExternalInput")
    dstl_d = nc.dram_tensor("dstl", [128, TOTBLKE], BF16, kind="ExternalInput")
    sidx_d = nc.dram_tensor("sidx", [128, TOTBLKE], I16, kind="ExternalInput")
    degp_d = nc.dram_tensor("degp", [128, TOTBLKE], F32, kind="ExternalInput")
    wgcn_d = nc.dram_tensor("wgcn", [D, D], F32, kind="ExternalInput")
    bgcn_d = nc.dram_tensor("bgcn", [D, 1], F32, kind="ExternalInput")
    wfc_d = nc.dram_tensor("wfc", [D, D], F32, kind="ExternalInput")
    bfc_d = nc.dram_tensor("bfc", [D, 1], F32, kind="ExternalInput")
    out_d = nc.dram_tensor("out", [D, GPC], F32, kind="ExternalOutput")

    # per-tile block ranges + max blocks in any bank-batch
    tile_blk0 = []
    pos = 0
    for t in range(TT):
        tile_blk0.append((pos, blocks[t][0], blocks[t][1]))
        pos += blocks[t][0] + blocks[t][1]
    tile_blk0.append((pos, 0, 0))
    GMAX = 0
    GMAXE = max(batch_nblk_e)
    for b0 in range(0, TT, BB):
        nb = min(BB, TT - b0)
        GMAX = max(GMAX, tile_blk0[b0 + nb][0] - tile_blk0[b0][0])

    with tile.TileContext(nc) as tc:
        with (
            tc.tile_pool(name="consts", bufs=1) as cp,
        ):
            # ---------------- constants / tables ----------------
            idx_t = cp.tile([128, TOTPOS // 16], I16)
            nc.sync.dma_start(out=idx_t[:], in_=idx_d[:])
            dstl_t = cp.tile([128, TOTBLKE], BF16)
            nc.sync.dma_start(out=dstl_t[:], in_=dstl_d[:])
            sidx_t = cp.tile([128, TOTBLKE], I16)
            nc.sync.dma_start(out=sidx_t[:], in_=sidx_d[:])
            degp_t = cp.tile([128, TOTBLKE], F32)
            nc.sync.dma_start(out=degp_t[:], in_=degp_d[:])
            wgcn_t = cp.tile([D, D], F32)
            nc.sync.dma_start(out=wgcn_t[:], in_=wgcn_d[:])
            bgcn_t = cp.tile([D, 1], F32)
            nc.sync.dma_start(out=bgcn_t[:], in_=bgcn_d[:])
            wfc_t = cp.tile([D, D], F32)
            nc.sync.dma_start(out=wfc_t[:], in_=wfc_d[:])
            bfc_t = cp.tile([D, 1], F32)
            nc.sync.dma_start(out=bfc_t[:], in_=bfc_d[:])

            iota_t = cp.tile([128, 64], BF16)
            nc.gpsimd.iota(iota_t[:], pattern=[[1, 64]], base=0,
                           channel_multiplier=0,
                           allow_small_or_imprecise_dtypes=True)
            ident_t = cp.tile([128, 128], F32)
            make_identity(nc, ident_t[:])
            if GSRC == "sbuf":
                # I(64) stacked twice so slices at base partition 0 and 64 both
                # see an identity (transpose operands must share base partition)
                identb_t = cp.tile([128, 64], BF16)
                make_identity(nc, identb_t[0:64, :])
                make_identity(nc, identb_t[64:128, :])

            # normv = (deg_src*deg_dst)^-1/2 in bf16
            rec_t = cp.tile([128, TOTBLKE], F32)
            nc.vector.reciprocal(rec_t[:], degp_t[:])
            nc.scalar.activation(rec_t[:], rec_t[:],
                                 mybir.ActivationFunctionType.Sqrt)
            normv_t = cp.tile([128, TOTBLKE], BF16)
            nc.vector.tensor_copy(out=normv_t[:], in_=rec_t[:])

            if GSRC == "sbuf":
                xsb_t = cp.tile([128, RNK * 2 * D], BF16)
                nc.sync.dma_start(out=xsb_t[:], in_=xsb_d[:])
            else:
                x_pairs = x_d[:].rearrange("(v two) d -> v (two d)", two=2)

            REPEAT = int(os.environ.get("GNN_REPEAT", "1"))
            STAGE = int(os.environ.get("GNN_STAGE", "9"))
            _QRR = [0]

            # ---- spectral norm sigma(W_fc) -> wfcT (iteration-invariant,
            # overlaps the gather phase instead of serializing after pooling)
            wfcT = cp.tile([D, D], F32)
            if STAGE >= 5:
                pf = tc.alloc_tile_pool(name="psum_fc", bufs=1, space="PSUM")
                mp = pf.tile([D, D], F32, tag="mp")
                nc.tensor.matmul(out=mp[:], lhsT=wfc_t[:], rhs=wfc_t[:],
                                 start=True, stop=True)
                m1_sb = cp.tile([D, D], F32, tag="m1sb")
                nc.scalar.copy(out=m1_sb[:], in_=mp[:])
                cur = m1_sb
                for _ in range(6):  # M^64
                    mp2 = pf.tile([D, D], F32, tag="mp")
                    nc.tensor.matmul(out=mp2[:], lhsT=cur[:], rhs=cur[:],
                                     start=True, stop=True)
                    nxt = cp.tile([D, D], F32, tag=f"m{_}")
                    nc.scalar.copy(out=nxt[:], in_=mp2[:])
                    cur = nxt
                ones_c = cp.tile([D, 1], F32)
                nc.vector.memset(ones_c[:], 1.0)
                ones_r = cp.tile([1, D], F32)
                nc.vector.memset(ones_r[:], 1.0)
                vp = pf.tile([D, 1], F32, tag="vp")
                nc.tensor.matmul(out=vp[:], lhsT=cur[:], rhs=ones_c[:],
                                 start=True, stop=True)
                v_sb = cp.tile([D, 1], F32)
                nc.scalar.copy(out=v_sb[:], in_=vp[:])
                wp = pf.tile([D, 1], F32, tag="vp")
                nc.tensor.matmul(out=wp[:], lhsT=m1_sb[:], rhs=v_sb[:],
                                 start=True, stop=True)
                w_sb = cp.tile([D, 1], F32)
                nc.scalar.copy(out=w_sb[:], in_=wp[:])
                nump = pf.tile([1, 1], F32, tag="sc")
                nc.tensor.matmul(out=nump[:], lhsT=v_sb[:], rhs=w_sb[:],
                                 start=True, stop=True)
                denp = pf.tile([1, 1], F32, tag="sc")
                nc.tensor.matmul(out=denp[:], lhsT=v_sb[:], rhs=v_sb[:],
                                 start=True, stop=True)
                num_sb = cp.tile([1, 1], F32, tag="num")
                den_sb = cp.tile([1, 1], F32, tag="den")
                nc.vector.tensor_copy(out=num_sb[:], in_=nump[:])
                nc.vector.tensor_copy(out=den_sb[:], in_=denp[:])
                rinv = cp.tile([1, 1], F32, tag="rinv")
                nc.vector.reciprocal(rinv[:], num_sb[:])
                nc.vector.tensor_tensor(out=rinv[:], in0=rinv[:], in1=den_sb[:],
                                        op=mybir.AluOpType.mult)
                nc.scalar.activation(rinv[:], rinv[:],
                                     mybir.ActivationFunctionType.Sqrt)
                sp = pf.tile([D, 1], F32, tag="vp")
                nc.tensor.matmul(out=sp[:], lhsT=ones_r[:], rhs=rinv[:],
                                 start=True, stop=True)
                s_col = cp.tile([D, 1], F32)
                nc.scalar.copy(out=s_col[:], in_=sp[:])

                # W_fc^T, scaled by 1/sigma
                wtp = pf.tile([D, D], F32, tag="mp")
                nc.tensor.transpose(out=wtp[:], in_=wfc_t[:],
                                    identity=ident_t[:D, :D])
                nc.vector.tensor_scalar_mul(wfcT[:], wtp[:], s_col[:])
                pf.release()
            for _it in range(REPEAT):
                hp = tc.alloc_tile_pool(name=f"h_{_it}", bufs=1)
                hT = hp.tile([64, S], F32)

                gp = tc.alloc_tile_pool(name=f"gath_{_it}", bufs=2)
                ohp = tc.alloc_tile_pool(name=f"oh_{_it}", bufs=2)
                tp = tc.alloc_tile_pool(name=f"tail_{_it}", bufs=2)
                pa = tc.alloc_tile_pool(name=f"psum_acc_{_it}", bufs=2, space="PSUM")
                pm = tc.alloc_tile_pool(name=f"psum_misc_{_it}", bufs=2, space="PSUM")
                tpp = (tc.alloc_tile_pool(name=f"psum_tr_{_it}", bufs=2,
                                          space="PSUM")
                       if GSRC == "sbuf" else None)
                for bi, b0 in enumerate(range(0, TT, BB) if STAGE >= 2 else []):
                    nb = min(BB, TT - b0)
                    blk0 = tile_blk0[b0][0]
                    blk1 = tile_blk0[b0 + nb][0] if b0 + nb < TT else TOTBLK
                    nblk = blk1 - blk0
                    nblk_e = batch_nblk_e[bi]
                    e0 = ebl0[bi]
                    if nblk > 0:
                        # class (parity) of each block in this batch
                        blkq = []
                        for t in range(b0, b0 + nb):
                            _, be_, bo_ = tile_blk0[t]
                            blkq += [0] * be_ + [1] * bo_
                        if GSRC == "sbuf":
                            gT = gp.tile([128, GMAX * 128], BF16, tag="gT")
                            for g0 in range(0, nblk, GSTEP):
                                gl = min(GSTEP, nblk - g0)
                                nc.gpsimd.dma_gather(
                                    out_ap=gT[:, g0 * 128:(g0 + gl) * 128]
                                    .rearrange("p (one n) -> p one n", one=1),
                                    in_ap=xsb_t[:],
                                    idxs_ap=idx_t[
                                        :, (blk0 + g0) * 8:(blk0 + g0 + gl) * 8],
                                    num_idxs=gl * 128,
                                    num_idxs_reg=gl * 128,
                                    elem_size=128,
                                    transpose=True,
                                    sbuf_tokens_per_rank=128,
                                    sbuf_free_dim_per_rank=2 * D * 2,
                                    queue_num=_QRR[0],
                                )
                                _QRR[0] = (_QRR[0] + 1) % NQ
                            if STAGE < 3:
                                continue
                            # PE-transpose to edge-major, class half selected
                            gat = gp.tile([128, GMAX, 64], BF16, tag="gat")
                            for g0 in range(0, nblk, 8):
                                gl = min(8, nblk - g0)
                                trg = tpp.tile([128, 8, 64], BF16, tag="trg")
                                for j in range(gl):
                                    lb = g0 + j
                                    q = blkq[lb]
                                    nc.tensor.transpose(
                                        out=trg[:, j, :],
                                        in_=gT[64 * q:64 * q + 64,
                                               lb * 128:(lb + 1) * 128],
                                        identity=identb_t[
                                            64 * q:64 * q + 64, :],
                                    )
                                nc.scalar.copy(out=gat[:, g0:g0 + gl, :],
                                               in_=trg[:, :gl, :])
                        else:
                            gat = gp.tile([128, GMAX, 128], BF16, tag="gat")
                            for g0 in range(0, nblk, GSTEP):
                                gl = min(GSTEP, nblk - g0)
                                nc.gpsimd.dma_gather(
                                    out_ap=gat[:, g0:g0 + gl, :],
                                    in_ap=x_pairs,
                                    idxs_ap=idx_t[
                                        :, (blk0 + g0) * 8:(blk0 + g0 + gl) * 8],
                                    num_idxs=gl * 128,
                                    num_idxs_reg=gl * 128,
                                    elem_size=128,
                                    single_packet=SPKT,
                                    queue_num=_QRR[0],
                                )
                                _QRR[0] = (_QRR[0] + 1) % NQ
                            if STAGE < 3:
                                continue
                        # valued one-hot [128, nblk_e, 64]
                        ohv = ohp.tile([128, GMAXE, 64], BF16, tag="ohv")
                        if bi < NDVE:
                            nc.vector.tensor_tensor(
                                out=ohv[:, :nblk_e, :],
                                in0=dstl_t[:, e0:e0 + nblk_e].to_broadcast(
                                    [128, nblk_e, 64]),
                                in1=iota_t[:, :64].rearrange(
                                    "p (j m) -> p j m", j=1).to_broadcast(
                                    [128, nblk_e, 64]),
                                op=mybir.AluOpType.is_equal,
                            )
                            nc.vector.tensor_tensor(
                                out=ohv[:, :nblk_e, :],
                                in0=ohv[:, :nblk_e, :],
                                in1=normv_t[:, e0:e0 + nblk_e].to_broadcast(
                                    [128, nblk_e, 64]),
                                op=mybir.AluOpType.mult,
                            )
                        else:
                            for c0 in range(0, nblk_e, CH):
                                ch = min(CH, nblk_e - c0)
                                nc.gpsimd.local_scatter(
                                    out_ap=ohv[:, c0:c0 + ch, :].rearrange(
                                        "p a b -> p (a b)"),
                                    data_ap=normv_t[:, e0 + c0:e0 + c0 + ch],
                                    idxs_ap=sidx_t[:, e0 + c0:e0 + c0 + ch],
                                    channels=128,
                                    num_elems=ch * 64,
                                    num_idxs=ch,
                                )
                    acc = pa.tile([64, BB, 64], F32, tag="acc")
                    if nblk == 0:
                        nc.vector.memset(acc[:], 0.0)
                    for ti in range(nb):
                        t = b0 + ti
                        base, be, bo = tile_blk0[t]
                        nblks_t = be + bo
                        first = True
                        for j in range(nblks_t):
                            q = 0 if j < be else 1
                            jj = j if j < be else j - be
                            k = 128
                            if jj == (be if q == 0 else bo) - 1:
                                k = kmax[t][q] - 128 * jj
                                k = 128 if k <= 0 else k
                            lb = base - blk0 + j
                            rhs = (gat[:k, lb, :] if GSRC == "sbuf"
                                   else gat[:k, lb, 64 * q:64 * q + 64])
                            nc.tensor.matmul(
                                out=acc[:, ti, :],
                                lhsT=ohv[:k, lb, :],
                                rhs=rhs,
                                start=first,
                                stop=(j == nblks_t - 1),
                            )
                            first = False
                        if nblks_t == 0 and nblk > 0:
                            nc.vector.memset(acc[:, ti, :], 0.0)

                    if STAGE < 4:
                        continue
                    # tail per batch: transpose, transform, bias + leaky
                    agg_sb = tp.tile([64, BB, 64], F32, tag="aggsb")
                    nc.scalar.copy(out=agg_sb[:, :nb, :], in_=acc[:, :nb, :])
                    trp = pm.tile([64, BB, 64], F32, tag="trp")
                    for ti in range(nb):
                        nc.tensor.transpose(
                            out=trp[:, ti, :], in_=agg_sb[:, ti, :],
                            identity=ident_t[:64, :64],
                        )
                    aggT = tp.tile([64, BB, 64], F32, tag="aggT")
                    nc.scalar.copy(out=aggT[:, :nb, :], in_=trp[:, :nb, :])
                    hps = pm.tile([64, BB * 64], F32, tag="hps")
                    nc.tensor.matmul(
                        out=hps[:, :nb * 64],
                        lhsT=wgcn_t[:],
                        rhs=aggT[:, :nb, :].rearrange("p a b -> p (a b)"),
                        start=True, stop=True,
                    )
                    if LRELU:
                        # fused bias + leaky_relu(0.2) on the Activation engine
                        nc.scalar.activation(
                            out=hT[:, b0 * 64:(b0 + nb) * 64],
                            in_=hps[:, :nb * 64],
                            func=mybir.ActivationFunctionType.Lrelu,
                            bias=bgcn_t[:], alpha=0.2,
                        )
                    else:
                        nc.scalar.activation(
                            out=hT[:, b0 * 64:(b0 + nb) * 64], in_=hps[:, :nb * 64],
                            func=mybir.ActivationFunctionType.Identity,
                            bias=bgcn_t[:],
                        )
                        # leaky relu on this batch's columns
                        lk = tp.tile([64, BB * 64], F32, tag="lk")
                        nc.vector.tensor_scalar_mul(
                            lk[:, :nb * 64], hT[:, b0 * 64:(b0 + nb) * 64], 0.2)
                        nc.vector.tensor_tensor(
                            out=hT[:, b0 * 64:(b0 + nb) * 64],
                            in0=hT[:, b0 * 64:(b0 + nb) * 64],
                            in1=lk[:, :nb * 64], op=mybir.AluOpType.max)

                for _pool in ((tpp,) if tpp else ()) + (pm, pa, tp, ohp, gp):
                    _pool.release()

                # ---------------- pooling ----------------
                pooledT = hp.tile([64, GPC], F32)
                for g in (range(GPC) if STAGE >= 5 else []):
                    nc.vector.tensor_reduce(
                        out=pooledT[:, g:g + 1],
                        in_=hT[:, g * GCAP:(g + 1) * GCAP],
                        axis=mybir.AxisListType.X,
                        op=mybir.AluOpType.max,
                    )

                # ---------------- FC apply: outT = (W/sigma) @ pooledT + b_fc
                out_sb0 = cp.tile([D, GPC], F32, tag="outsb0")
                if STAGE >= 5:
                    pfa = tc.alloc_tile_pool(name=f"psum_fca_{_it}", bufs=1,
                                             space="PSUM")
                    op_ = pfa.tile([D, GPC], F32, tag="op")
                    nc.tensor.matmul(out=op_[:], lhsT=wfcT[:], rhs=pooledT[:],
                                     start=True, stop=True)
                    out_sb = cp.tile([D, GPC], F32)
                    nc.scalar.activation(out=out_sb[:], in_=op_[:],
                                         func=mybir.ActivationFunctionType.Identity,
                                         bias=bfc_t[:])
                    nc.sync.dma_start(out=out_d[:], in_=out_sb[:])
                    pfa.release()
                else:
                    nc.vector.memset(out_sb0[:], 0.0)
                    nc.sync.dma_start(out=out_d[:], in_=out_sb0[:])
                hp.release()


    nc.compile()
    return nc


# ----------------------------------------------------------------------------
# Cached executor (compile once per dims signature)
# ----------------------------------------------------------------------------
_CACHE = {}


class _Exec:
    def __init__(self, dims):
        self.dims = dims
        self.nc = _build_program(dims)

    def run(self, in_maps):
        from concourse.bass_utils import run_bass_kernel_spmd
        res = run_bass_kernel_spmd(self.nc, in_maps, list(range(NCORES)))
        return [r["out"] for r in res.results]


def _get_exec(dims):
    key = repr(sorted(dims.items()))
    if key not in _CACHE:
        _CACHE[key] = _Exec(dims)
    return _CACHE[key]


def _make_in_maps(dims, tables, W_gcn, b_gcn, W_fc, b_fc):
    wgcn = np.asarray(W_gcn, dtype=np.float32)
    bgcn = np.asarray(b_gcn, dtype=np.float32).reshape(-1, 1)
    wfc = np.asarray(W_fc, dtype=np.float32)
    bfc = np.asarray(b_fc, dtype=np.float32).reshape(-1, 1)
    gsrc = os.environ.get("GNN_GSRC", "hbm")
    xin = ({"xsb": tables["x_sb"]} if gsrc == "sbuf"
           else {"x": tables["x_pad"]})
    in_maps = []
    for c in range(NCORES):
        in_maps.append({
            **xin,
            "idx": np.ascontiguousarray(tables["idx"][c]),
            "dstl": np.ascontiguousarray(tables["dstl"][c]),
            "sidx": np.ascontiguousarray(tables["sidx"][c]),
            "degp": np.ascontiguousarray(tables["degp"][c]),
            "wgcn": wgcn,
            "bgcn": bgcn,
            "wfc": wfc,
            "bfc": bfc,
        })
    return in_maps


def kernel(x, W_gcn, b_gcn, W_fc, b_fc, edge_index, batch, num_graphs):
    dims, tables = _preprocess(x, edge_index, batch, num_graphs)
    ex = _get_exec(dims)
    in_maps = _make_in_maps(dims, tables, W_gcn, b_gcn, W_fc, b_fc)
    outs = ex.run(in_maps)
    B = dims["B"]
    D = dims["D"]
    result = np.zeros((B, D), dtype=np.float32)
    for c in range(NCORES):
        o = np.asarray(outs[c], dtype=np.float32)  # [D, GPC]
        for gi, g in enumerate(tables["core_graphs"][c]):
            if g >= 0:
                result[g] = o[:, gi]
    return result


# ----------------------------------------------------------------------------
# Reusable jitted runner (for steady-state timing): mirrors
# bass2jax.run_bass_via_pjrt's multi-core path but keeps the jitted callable.
# ----------------------------------------------------------------------------
def _build_jit(nc):
    import jax
    import numpy as _np
    from jax.sharding import Mesh, PartitionSpec
    from jax.experimental.shard_map import shard_map
    from concourse import bass2jax
    from concourse import mybir as _mb

    bass2jax.install_neuronx_cc_hook()
    in_names, out_names, out_avals, zero_outs = [], [], [], []
    partition_name = (nc.partition_id_tensor.name
                      if nc.partition_id_tensor else None)
    for alloc in nc.m.functions[0].allocations:
        if not isinstance(alloc, _mb.MemoryLocationSet):
            continue
        name = alloc.memorylocations[0].name
        if alloc.kind == "ExternalInput":
            if name != partition_name:
                in_names.append(name)
        elif alloc.kind == "ExternalOutput":
            out_names.append(name)
            shape = tuple(alloc.tensor_shape)
            dtype = _mb.dt.np(alloc.dtype)
            out_avals.append(jax.core.ShapedArray(shape, dtype))
            zero_outs.append(_np.zeros(shape, dtype))
    n_params = len(in_names)
    all_in = list(in_names) + list(out_names)
    if partition_name is not None:
        all_in.append(partition_name)

    def _body(*args):
        operands = list(args)
        if partition_name is not None:
            operands.append(bass2jax.partition_id_tensor())
        outs = bass2jax._bass_exec_p.bind(
            *operands,
            out_avals=tuple(out_avals),
            in_names=tuple(all_in),
            out_names=tuple(out_names),
            lowering_input_output_aliases=(),
            sim_require_finite=True,
            sim_require_nnan=True,
            nc=nc,
        )
        return tuple(outs)

    devices = jax.devices()[:NCORES]
    mesh = Mesh(np.asarray(devices), ("core",))
    in_specs = (PartitionSpec("core"),) * (n_params + len(out_names))
    out_specs = (PartitionSpec("core"),) * len(out_names)
    donate = tuple(range(n_params, n_params + len(out_names)))
    fn = jax.jit(
        shard_map(_body, mesh=mesh, in_specs=in_specs, out_specs=out_specs,
                  check_rep=False),
        donate_argnums=donate, keep_unused=True,
    )
    return fn, in_names, out_names, zero_outs


def time_exec(ex, in_maps, reps=24, warmup=4):
    """Median per-iteration device time via back-to-back dispatch."""
    import jax
    import time as _t
    fn, in_names, out_names, zero_outs = _build_jit(ex.nc)
    concat = [np.concatenate([np.asarray(in_maps[c][n]) for c in range(NCORES)],
                             axis=0) for n in in_names]
    dev_in = [jax.device_put(a) for a in concat]
    for a in dev_in:
        a.block_until_ready()

    def zouts():
        return [np.concatenate([z] * NCORES, axis=0) for z in zero_outs]

    def run_n(n):
        outs = None
        t0 = _t.perf_counter()
        for _ in range(n):
            outs = fn(*dev_in, *zouts())
        for o in outs:
            o.block_until_ready()
        return _t.perf_counter() - t0

    run_n(warmup)
    t1 = run_n(reps // 2)
    t2 = run_n(reps)
    per_iter = (t2 - t1) / (reps - reps // 2)
    return per_iter * 1e9


# revision 31
# speedup vs baseline: 1.1341x; 1.1341x over previous
"""GCNHead Trainium2 kernel (8-core SPMD), v2.

Math (matches reference):
  deg = bincount(dst)+1 (self loops);  dinv = deg^-1/2
  agg[n] = sum_{e: dst=n} dinv[src]*dinv[dst] * x[src]   (+ self loop)
  h = agg[n] @ W_gcn + b_gcn
  out = leaky_relu(h, 0.2);  pooled = segment_max(out, batch)
  result = pooled @ (W_fc / sigma_max(W_fc)).T + b_fc

v2 design: the edge norm dinv[src]*dinv[dst] is folded into a VALUED one-hot
(entry = norm instead of 1), so the x~ prescale pass of v1 disappears and the
gather reads raw bf16 x directly (saves a full read+write of x and removes the
serial prefix before gathering can start).  The valued one-hot is built on DVE
(is_equal vs iota, then mult by the norm table; GNN_NDVE batches) with a
gpsimd local_scatter fallback for the rest -- NDVE=14 (all DVE) measured
fastest since Pool is saturated by gather descriptor generation.  The
spectral-norm sigma(W_fc) chain is hoisted out of the repeat loop (it is a
~40-op serial dependency chain that otherwise adds ~100us of semaphore
latency after pooling); bias+leaky_relu is fused into one Activation-engine
Lrelu op.  Device-verified limits: dma_gather calls are capped at 1024
descriptors; SBUF-source dma_gather is not supported by this runtime.

Host does integer-only preprocessing (sharding, bucketing, index tables,
degree products via bincount); all float math runs on device.
"""
import sys

sys.path.insert(0, "/opt/trn_rl_repo")

import math
import os
import numpy as np
import ml_dtypes

import concourse.bass as bass
import concourse.mybir as mybir
import concourse.tile as tile
from concourse import bacc
from concourse.masks import make_identity

BF16 = mybir.dt.bfloat16
F32 = mybir.dt.float32
I16 = mybir.dt.int16

NCORES = 8
SENT = 20000.0  # dst_local sentinel (never matches iota 0..63)
BB = 8          # tiles per bank-batch
CH = 30         # blocks per local_scatter call (30*64=1920 < 2048 elems)


# ----------------------------------------------------------------------------
# Host preprocessing (integers only)
# ----------------------------------------------------------------------------
def _preprocess(x, edge_index, batch, num_graphs):
    N, D = x.shape
    B = int(num_graphs)
    src = np.asarray(edge_index[0], dtype=np.int64)
    dst = np.asarray(edge_index[1], dtype=np.int64)
    batch = np.asarray(batch, dtype=np.int64)

    deg = np.bincount(dst, minlength=N).astype(np.int64) + 1  # + self loop

    # graph -> node range (batch sorted)
    counts_g = np.bincount(batch, minlength=B)
    starts_g = np.concatenate([[0], np.cumsum(counts_g)])

    GPC = math.ceil(B / NCORES)  # graphs per core
    # balance graphs across cores by total edge weight (snake over sorted)
    gw = np.add.reduceat(deg, starts_g[:-1]) if N else counts_g
    gw = np.where(counts_g > 0, gw, 0)
    order = np.argsort(-gw, kind="stable")
    core_graphs = [[] for _ in range(NCORES)]
    loads = np.zeros(NCORES)
    for g in order:
        c = int(np.argmin([loads[i] + (1e18 if len(core_graphs[i]) >= GPC else 0)
                           for i in range(NCORES)]))
        core_graphs[c].append(int(g))
        loads[c] += gw[g]
    for c in range(NCORES):
        core_graphs[c] += [-1] * (GPC - len(core_graphs[c]))

    GCAP = 64 * max(1, math.ceil(counts_g.max() / 64))
    TPG = GCAP // 64          # tiles per graph
    TT = GPC * TPG            # tiles per core
    S = TT * 64               # slots per core

    # --- slot assignment: per graph, balance node degree across TPG bins ---
    node_slot = np.full(N, -1, dtype=np.int64)   # slot within its core
    node_core = np.full(N, -1, dtype=np.int64)
    for c in range(NCORES):
        for gi, g in enumerate(core_graphs[c]):
            if g < 0:
                continue
            nodes = np.arange(starts_g[g], starts_g[g + 1])
            if len(nodes) == 0:
                continue
            nd = deg[nodes]
            ordn = np.argsort(-nd, kind="stable")
            binload = np.zeros(TPG, dtype=np.int64)
            binfill = np.zeros(TPG, dtype=np.int64)
            for i in ordn:
                masked = np.where(binfill < 64, binload, np.iinfo(np.int64).max)
                b = int(np.argmin(masked))
                slot = gi * GCAP + b * 64 + binfill[b]
                node_slot[nodes[i]] = slot
                node_core[nodes[i]] = c
                binfill[b] += 1
                binload[b] += nd[i]

    # --- edges (incl self loops) bucketed per (core, tile, parity) ---
    loop = np.arange(N, dtype=np.int64)
    esrc = np.concatenate([src, loop])
    edst = np.concatenate([dst, loop])
    ecore = node_core[edst]
    eslot = node_slot[edst]
    etile = eslot >> 6
    edl = (eslot & 63).astype(np.int64)
    epar = (esrc & 1).astype(np.int64)

    # per-core lists of (tile, parity) buckets; order edges by key
    counts = np.zeros((NCORES, TT, 2), dtype=np.int64)
    per_core_order = []
    for c in range(NCORES):
        sel = np.where(ecore == c)[0]
        k = etile[sel] * 2 + epar[sel]
        o = np.argsort(k, kind="stable")
        sel = sel[o]
        per_core_order.append(sel)
        cnt = np.bincount(k[o], minlength=TT * 2)
        counts[c] = cnt.reshape(TT, 2)

    # class capacity = max over cores, rounded to 128
    cap = ((counts.max(axis=0) + 127) // 128) * 128      # [TT, 2]
    blocks = cap // 128                                   # [TT, 2]
    TOTBLK = int(blocks.sum())
    TOTPOS = TOTBLK * 128

    # global block layout: tiles ascending, class even then odd
    class_off = np.zeros((TT, 2), dtype=np.int64)        # position offsets
    pos = 0
    for t in range(TT):
        for q in range(2):
            class_off[t, q] = pos
            pos += cap[t, q]

    # per-tile first block; bank-batch block spans (even-rounded for scatter)
    tile_blk0 = np.zeros(TT + 1, dtype=np.int64)
    for t in range(TT):
        tile_blk0[t + 1] = tile_blk0[t] + blocks[t, 0] + blocks[t, 1]
    nbatch = math.ceil(TT / BB)
    batch_blk0 = np.array([tile_blk0[min(b * BB, TT)] for b in range(nbatch + 1)])
    batch_nblk = np.diff(batch_blk0)
    batch_nblk_e = (batch_nblk + 1) // 2 * 2              # even-rounded
    ebl0 = np.concatenate([[0], np.cumsum(batch_nblk_e)])
    TOTBLKE = int(ebl0[-1])

    # tables
    idx_tab = np.zeros((NCORES, TOTPOS), dtype=np.int64)
    dstl_tab = np.full((NCORES, 128, TOTBLKE), SENT, dtype=np.float32)
    sidx_tab = np.full((NCORES, 128, TOTBLKE), -1, dtype=np.int16)
    degp_tab = np.ones((NCORES, 128, TOTBLKE), dtype=np.float32)
    blk_batch = np.searchsorted(batch_blk0[1:], np.arange(TOTBLK), side="right")
    blk_ecol = ebl0[blk_batch] + (np.arange(TOTBLK) - batch_blk0[blk_batch])
    blk_local = np.arange(TOTBLK) - batch_blk0[blk_batch]
    for c in range(NCORES):
        sel = per_core_order[c]
        k = etile[sel] * 2 + epar[sel]
        # position within class = running index per class
        cstart = np.concatenate([[0], np.cumsum(np.bincount(k, minlength=TT * 2))])
        within = np.arange(len(sel)) - cstart[k]
        gpos = class_off.reshape(-1)[k] + within
        idx_tab[c, gpos] = esrc[sel] >> 1
        blk = gpos >> 7
        lane = gpos & 127
        ecol = blk_ecol[blk]
        dstl_tab[c, lane, ecol] = edl[sel]
        sidx_tab[c, lane, ecol] = (blk_local[blk] % CH) * 64 + edl[sel]
        degp_tab[c, lane, ecol] = deg[esrc[sel]] * deg[edst[sel]]

    # idx table SBUF layout [128, TOTPOS/16]: flat i -> [i%16 (+16r), i//16]
    idx16 = idx_tab.astype(np.int16).reshape(NCORES, TOTPOS // 16, 16)
    idx16 = np.ascontiguousarray(idx16.transpose(0, 2, 1))           # [NC,16,P/16]
    idx128 = np.tile(idx16, (1, 8, 1))                               # [NC,128,...]
    dstl128 = dstl_tab.astype(ml_dtypes.bfloat16)

    NP = ((N + 255) // 256) * 256          # pad to even multiple of 128
    x_pad = np.zeros((NP, D), dtype=ml_dtypes.bfloat16)
    x_pad[:N] = np.asarray(x, dtype=np.float32).astype(ml_dtypes.bfloat16)
    # SBUF-resident pair layout: pair p -> partition p%128, rank p//128
    NPAIR = NP // 2
    RNK = NPAIR // 128
    x_sb = np.ascontiguousarray(
        x_pad.reshape(RNK, 128, 2 * D).transpose(1, 0, 2)
    ).reshape(128, RNK * 2 * D)

    dims = dict(N=N, D=D, B=B, GPC=GPC, GCAP=GCAP, TPG=TPG, TT=TT, S=S, NP=NP,
                TOTBLK=TOTBLK, TOTPOS=TOTPOS, TOTBLKE=TOTBLKE,
                blocks=tuple(map(tuple, blocks)),
                kmax=tuple(map(tuple, counts.max(axis=0))),
                batch_nblk_e=tuple(batch_nblk_e), ebl0=tuple(ebl0))
    tables = dict(idx=idx128, dstl=dstl128, sidx=sidx_tab, degp=degp_tab,
                  x_pad=x_pad, x_sb=x_sb, core_graphs=core_graphs)
    return dims, tables


# ----------------------------------------------------------------------------
# Device program
# ----------------------------------------------------------------------------
def _build_program(dims):
    D = dims["D"]
    TT, TPG, GPC, GCAP = dims["TT"], dims["TPG"], dims["GPC"], dims["GCAP"]
    NP, TOTBLK, TOTPOS = dims["NP"], dims["TOTBLK"], dims["TOTPOS"]
    TOTBLKE = dims["TOTBLKE"]
    blocks = dims["blocks"]
    kmax = dims["kmax"]
    batch_nblk_e = dims["batch_nblk_e"]
    ebl0 = dims["ebl0"]
    S = dims["S"]

    SCRATCH = int(os.environ.get("GNN_SCRATCH", "16384"))
    GSTEP = int(os.environ.get("GNN_GSTEP", "8"))         # blocks per gather
    # NOTE: the device rejects gather calls over 1024 descriptors (GSTEP>8).
    NDVE = int(os.environ.get("GNN_NDVE", "14"))          # batches on DVE path
    NQ = int(os.environ.get("GNN_NQ", "4"))
    LRELU = int(os.environ.get("GNN_LRELU", "1"))
    GSRC = os.environ.get("GNN_GSRC", "hbm")              # hbm | sbuf
    SPKT = bool(int(os.environ.get("GNN_SPKT", "1")))

    nc = bacc.Bacc("TRN2", target_bir_lowering=False, debug=False,
                   num_swdge_queues=NQ, dynamic_dma_scratch_size=SCRATCH)
    if GSRC == "sbuf":
        RNK = (NP // 2) // 128
        xsb_d = nc.dram_tensor("xsb", [128, RNK * 2 * D], BF16,
                               kind="ExternalInput")
    else:
        x_d = nc.dram_tensor("x", [NP, D], BF16, kind="ExternalInput")
    idx_d = nc.dram_tensor("idx", [128, TOTPOS // 16], I16, kind="ExternalInput")
    dstl_d = nc.dram_tensor("dstl", [128, TOTBLKE], BF16, kind="ExternalInput")
    sidx_d = nc.dram_tensor("sidx", [128, TOTBLKE], I16, kind="ExternalInput")
    degp_d = nc.dram_tensor("degp", [128, TOTBLKE], F32, kind="ExternalInput")
    wgcn_d = nc.dram_tensor("wgcn", [D, D], F32, kind="ExternalInput")
    bgcn_d = nc.dram_tensor("bgcn", [D, 1], F32, kind="ExternalInput")
    wfc_d = nc.dram_tensor("wfc", [D, D], F32, kind="ExternalInput")
    bfc_d = nc.dram_tensor("bfc", [D, 1], F32, kind="ExternalInput")
    out_d = nc.dram_tensor("out", [D, GPC], F32, kind="ExternalOutput")

    # per-tile block ranges + max blocks in any bank-batch
    tile_blk0 = []
    pos = 0
    for t in range(TT):
        tile_blk0.append((pos, blocks[t][0], blocks[t][1]))
        pos += blocks[t][0] + blocks[t][1]
    tile_blk0.append((pos, 0, 0))
    GMAX = 0
    GMAXE = max(batch_nblk_e)
    for b0 in range(0, TT, BB):
        nb = min(BB, TT - b0)
        GMAX = max(GMAX, tile_blk0[b0 + nb][0] - tile_blk0[b0][0])

    with tile.TileContext(nc) as tc:
        with (
            tc.tile_pool(name="consts", bufs=1) as cp,
        ):
            # ---------------- constants / tables ----------------
            idx_t = cp.tile([128, TOTPOS // 16], I16)
            nc.sync.dma_start(out=idx_t[:], in_=idx_d[:])
            dstl_t = cp.tile([128, TOTBLKE], BF16)
            nc.sync.dma_start(out=dstl_t[:], in_=dstl_d[:])
            sidx_t = cp.tile([128, TOTBLKE], I16)
            nc.sync.dma_start(out=sidx_t[:], in_=sidx_d[:])
            degp_t = cp.tile([128, TOTBLKE], F32)
            nc.sync.dma_start(out=degp_t[:], in_=degp_d[:])
            wgcn_t = cp.tile([D, D], F32)
            nc.sync.dma_start(out=wgcn_t[:], in_=wgcn_d[:])
            bgcn_t = cp.tile([D, 1], F32)
            nc.sync.dma_start(out=bgcn_t[:], in_=bgcn_d[:])
            wfc_t = cp.tile([D, D], F32)
            nc.sync.dma_start(out=wfc_t[:], in_=wfc_d[:])
            bfc_t = cp.tile([D, 1], F32)
            nc.sync.dma_start(out=bfc_t[:], in_=bfc_d[:])

            iota_t = cp.tile([128, 64], BF16)
            nc.gpsimd.iota(iota_t[:], pattern=[[1, 64]], base=0,
                           channel_multiplier=0,
                           allow_small_or_imprecise_dtypes=True)
            ident_t = cp.tile([128, 128], F32)
            make_identity(nc, ident_t[:])
            if GSRC == "sbuf":
                # I(64) stacked twice so slices at base partition 0 and 64 both
                # see an identity (transpose operands must share base partition)
                identb_t = cp.tile([128, 64], BF16)
                make_identity(nc, identb_t[0:64, :])
                make_identity(nc, identb_t[64:128, :])

            # normv = (deg_src*deg_dst)^-1/2 in bf16
            rec_t = cp.tile([128, TOTBLKE], F32)
            nc.vector.reciprocal(rec_t[:], degp_t[:])
            nc.scalar.activation(rec_t[:], rec_t[:],
                                 mybir.ActivationFunctionType.Sqrt)
            normv_t = cp.tile([128, TOTBLKE], BF16)
            nc.vector.tensor_copy(out=normv_t[:], in_=rec_t[:])

            if GSRC == "sbuf":
                xsb_t = cp.tile([128, RNK * 2 * D], BF16)
                nc.sync.dma_start(out=xsb_t[:], in_=xsb_d[:])
            else:
                x_pairs = x_d[:].rearrange("(v two) d -> v (two d)", two=2)

            REPEAT = int(os.environ.get("GNN_REPEAT", "1"))
            STAGE = int(os.environ.get("GNN_STAGE", "9"))
            _QRR = [0]

            # ---- spectral norm sigma(W_fc) -> wfcT (iteration-invariant,
            # overlaps the gather phase instead of serializing after pooling)
            wfcT = cp.tile([D, D], F32)
            if STAGE >= 5:
                pf = tc.alloc_tile_pool(name="psum_fc", bufs=1, space="PSUM")
                mp = pf.tile([D, D], F32, tag="mp")
                nc.tensor.matmul(out=mp[:], lhsT=wfc_t[:], rhs=wfc_t[:],
                                 start=True, stop=True)
                m1_sb = cp.tile([D, D], F32, tag="m1sb")
                nc.scalar.copy(out=m1_sb[:], in_=mp[:])
                cur = m1_sb
                for _ in range(6):  # M^64
                    mp2 = pf.tile([D, D], F32, tag="mp")
                    nc.tensor.matmul(out=mp2[:], lhsT=cur[:], rhs=cur[:],
                                     start=True, stop=True)
                    nxt = cp.tile([D, D], F32, tag=f"m{_}")
                    nc.scalar.copy(out=nxt[:], in_=mp2[:])
                    cur = nxt
                ones_c = cp.tile([D, 1], F32)
                nc.vector.memset(ones_c[:], 1.0)
                ones_r = cp.tile([1, D], F32)
                nc.vector.memset(ones_r[:], 1.0)
                vp = pf.tile([D, 1], F32, tag="vp")
                nc.tensor.matmul(out=vp[:], lhsT=cur[:], rhs=ones_c[:],
                                 start=True, stop=True)
                v_sb = cp.tile([D, 1], F32)
                nc.scalar.copy(out=v_sb[:], in_=vp[:])
                wp = pf.tile([D, 1], F32, tag="vp")
                nc.tensor.matmul(out=wp[:], lhsT=m1_sb[:], rhs=v_sb[:],
                                 start=True, stop=True)
                w_sb = cp.tile([D, 1], F32)
                nc.scalar.copy(out=w_sb[:], in_=wp[:])
                nump = pf.tile([1, 1], F32, tag="sc")
                nc.tensor.matmul(out=nump[:], lhsT=v_sb[:], rhs=w_sb[:],
                                 start=True, stop=True)
                denp = pf.tile([1, 1], F32, tag="sc")
                nc.tensor.matmul(out=denp[:], lhsT=v_sb[:], rhs=v_sb[:],
                                 start=True, stop=True)
                num_sb = cp.tile([1, 1], F32, tag="num")
                den_sb = cp.tile([1, 1], F32, tag="den")
                nc.vector.tensor_copy(out=num_sb[:], in_=nump[:])
                nc.vector.tensor_copy(out=den_sb[:], in_=denp[:])
                rinv = cp.tile([1, 1], F32, tag="rinv")
                nc.vector.reciprocal(rinv[:], num_sb[:])
                nc.vector.tensor_tensor(out=rinv[:], in0=rinv[:], in1=den_sb[:],
                                        op=mybir.AluOpType.mult)
                nc.scalar.activation(rinv[:], rinv[:],
                                     mybir.ActivationFunctionType.Sqrt)
                sp = pf.tile([D, 1], F32, tag="vp")
                nc.tensor.matmul(out=sp[:], lhsT=ones_r[:], rhs=rinv[:],
                                 start=True, stop=True)
                s_col = cp.tile([D, 1], F32)
                nc.scalar.copy(out=s_col[:], in_=sp[:])

                # W_fc^T, scaled by 1/sigma
                wtp = pf.tile([D, D], F32, tag="mp")
                nc.tensor.transpose(out=wtp[:], in_=wfc_t[:],
                                    identity=ident_t[:D, :D])
                nc.vector.tensor_scalar_mul(wfcT[:], wtp[:], s_col[:])
                pf.release()
            for _it in range(REPEAT):
                hp = tc.alloc_tile_pool(name=f"h_{_it}", bufs=1)
                hT = hp.tile([64, S], F32)

                gp = tc.alloc_tile_pool(name=f"gath_{_it}", bufs=2)
                ohp = tc.alloc_tile_pool(name=f"oh_{_it}", bufs=2)
                tp = tc.alloc_tile_pool(name=f"tail_{_it}", bufs=2)
                pa = tc.alloc_tile_pool(name=f"psum_acc_{_it}", bufs=2, space="PSUM")
                pm = tc.alloc_tile_pool(name=f"psum_misc_{_it}", bufs=2, space="PSUM")
                tpp = (tc.alloc_tile_pool(name=f"psum_tr_{_it}", bufs=2,
                                          space="PSUM")
                       if GSRC == "sbuf" else None)
                for bi, b0 in enumerate(range(0, TT, BB) if STAGE >= 2 else []):
                    nb = min(BB, TT - b0)
                    blk0 = tile_blk0[b0][0]
                    blk1 = tile_blk0[b0 + nb][0] if b0 + nb < TT else TOTBLK
                    nblk = blk1 - blk0
                    nblk_e = batch_nblk_e[bi]
                    e0 = ebl0[bi]
                    if nblk > 0:
                        # class (parity) of each block in this batch
                        blkq = []
                        for t in range(b0, b0 + nb):
                            _, be_, bo_ = tile_blk0[t]
                            blkq += [0] * be_ + [1] * bo_
                        if GSRC == "sbuf":
                            gT = gp.tile([128, GMAX * 128], BF16, tag="gT")
                            for g0 in range(0, nblk, GSTEP):
                                gl = min(GSTEP, nblk - g0)
                                nc.gpsimd.dma_gather(
                                    out_ap=gT[:, g0 * 128:(g0 + gl) * 128]
                                    .rearrange("p (one n) -> p one n", one=1),
                                    in_ap=xsb_t[:],
                                    idxs_ap=idx_t[
                                        :, (blk0 + g0) * 8:(blk0 + g0 + gl) * 8],
                                    num_idxs=gl * 128,
                                    num_idxs_reg=gl * 128,
                                    elem_size=128,
                                    transpose=True,
                                    sbuf_tokens_per_rank=128,
                                    sbuf_free_dim_per_rank=2 * D * 2,
                                    queue_num=_QRR[0],
                                )
                                _QRR[0] = (_QRR[0] + 1) % NQ
                            if STAGE < 3:
                                continue
                            # PE-transpose to edge-major, class half selected
                            gat = gp.tile([128, GMAX, 64], BF16, tag="gat")
                            for g0 in range(0, nblk, 8):
                                gl = min(8, nblk - g0)
                                trg = tpp.tile([128, 8, 64], BF16, tag="trg")
                                for j in range(gl):
                                    lb = g0 + j
                                    q = blkq[lb]
                                    nc.tensor.transpose(
                                        out=trg[:, j, :],
                                        in_=gT[64 * q:64 * q + 64,
                                               lb * 128:(lb + 1) * 128],
                                        identity=identb_t[
                                            64 * q:64 * q + 64, :],
                                    )
                                nc.scalar.copy(out=gat[:, g0:g0 + gl, :],
                                               in_=trg[:, :gl, :])
                        else:
                            gat = gp.tile([128, GMAX, 128], BF16, tag="gat")
                            for g0 in range(0, nblk, GSTEP):
                                gl = min(GSTEP, nblk - g0)
                                nc.gpsimd.dma_gather(
                                    out_ap=gat[:, g0:g0 + gl, :],
                                    in_ap=x_pairs,
                                    idxs_ap=idx_t[
                                        :, (blk0 + g0) * 8:(blk0 + g0 + gl) * 8],
                                    num_idxs=gl * 128,
                                    num_idxs_reg=gl * 128,
                                    elem_size=128,
                                    single_packet=SPKT,
                                    queue_num=_QRR[0],
                                )
                                _QRR[0] = (_QRR[0] + 1) % NQ
                            if STAGE < 3:
                                continue
                        # valued one-hot [128, nblk_e, 64]
                        ohv = ohp.tile([128, GMAXE, 64], BF16, tag="ohv")
                        if bi < NDVE:
                            nc.vector.tensor_tensor(
                                out=ohv[:, :nblk_e, :],
                                in0=dstl_t[:, e0:e0 + nblk_e].to_broadcast(
                                    [128, nblk_e, 64]),
                                in1=iota_t[:, :64].rearrange(
                                    "p (j m) -> p j m", j=1).to_broadcast(
                                    [128, nblk_e, 64]),
                                op=mybir.AluOpType.is_equal,
                            )
                            nc.vector.tensor_tensor(
                                out=ohv[:, :nblk_e, :],
                                in0=ohv[:, :nblk_e, :],
                                in1=normv_t[:, e0:e0 + nblk_e].to_broadcast(
                                    [128, nblk_e, 64]),
                                op=mybir.AluOpType.mult,
                            )
                        else:
                            for c0 in range(0, nblk_e, CH):
                                ch = min(CH, nblk_e - c0)
                                nc.gpsimd.local_scatter(
                                    out_ap=ohv[:, c0:c0 + ch, :].rearrange(
                                        "p a b -> p (a b)"),
                                    data_ap=normv_t[:, e0 + c0:e0 + c0 + ch],
                                    idxs_ap=sidx_t[:, e0 + c0:e0 + c0 + ch],
                                    channels=128,
                                    num_elems=ch * 64,
                                    num_idxs=ch,
                                )
                    acc = pa.tile([64, BB, 64], F32, tag="acc")
                    if nblk == 0:
                        nc.vector.memset(acc[:], 0.0)
                    for ti in range(nb):
                        t = b0 + ti
                        base, be, bo = tile_blk0[t]
                        nblks_t = be + bo
                        first = True
                        for j in range(nblks_t):
                            q = 0 if j < be else 1
                            jj = j if j < be else j - be
                            k = 128
                            if jj == (be if q == 0 else bo) - 1:
                                k = kmax[t][q] - 128 * jj
                                k = 128 if k <= 0 else k
                            lb = base - blk0 + j
                            xop = (gat[:k, lb, :] if GSRC == "sbuf"
                                   else gat[:k, lb, 64 * q:64 * q + 64])
                            # lhsT = gathered x -> acc comes out [dim, slot],
                            # already transposed for the W_gcn transform
                            nc.tensor.matmul(
                                out=acc[:, ti, :],
                                lhsT=xop,
                                rhs=ohv[:k, lb, :],
                                start=first,
                                stop=(j == nblks_t - 1),
                            )
                            first = False
                        if nblks_t == 0 and nblk > 0:
                            nc.vector.memset(acc[:, ti, :], 0.0)

                    if STAGE < 4:
                        continue
                    # tail per batch: acc is already [dim, slot]; one copy out
                    # of PSUM, then the W_gcn transform
                    aggT = tp.tile([64, BB, 64], F32, tag="aggT")
                    nc.scalar.copy(out=aggT[:, :nb, :], in_=acc[:, :nb, :])
                    hps = pm.tile([64, BB * 64], F32, tag="hps")
                    nc.tensor.matmul(
                        out=hps[:, :nb * 64],
                        lhsT=wgcn_t[:],
                        rhs=aggT[:, :nb, :].rearrange("p a b -> p (a b)"),
                        start=True, stop=True,
                    )
                    if LRELU:
                        # fused bias + leaky_relu(0.2) on the Activation engine
                        nc.scalar.activation(
                            out=hT[:, b0 * 64:(b0 + nb) * 64],
                            in_=hps[:, :nb * 64],
                            func=mybir.ActivationFunctionType.Lrelu,
                            bias=bgcn_t[:], alpha=0.2,
                        )
                    else:
                        nc.scalar.activation(
                            out=hT[:, b0 * 64:(b0 + nb) * 64], in_=hps[:, :nb * 64],
                            func=mybir.ActivationFunctionType.Identity,
                            bias=bgcn_t[:],
                        )
                        # leaky relu on this batch's columns
                        lk = tp.tile([64, BB * 64], F32, tag="lk")
                        nc.vector.tensor_scalar_mul(
                            lk[:, :nb * 64], hT[:, b0 * 64:(b0 + nb) * 64], 0.2)
                        nc.vector.tensor_tensor(
                            out=hT[:, b0 * 64:(b0 + nb) * 64],
                            in0=hT[:, b0 * 64:(b0 + nb) * 64],
                            in1=lk[:, :nb * 64], op=mybir.AluOpType.max)

                for _pool in ((tpp,) if tpp else ()) + (pm, pa, tp, ohp, gp):
                    _pool.release()

                # ---------------- pooling ----------------
                pooledT = hp.tile([64, GPC], F32)
                for g in (range(GPC) if STAGE >= 5 else []):
                    nc.vector.tensor_reduce(
                        out=pooledT[:, g:g + 1],
                        in_=hT[:, g * GCAP:(g + 1) * GCAP],
                        axis=mybir.AxisListType.X,
                        op=mybir.AluOpType.max,
                    )

                # ---------------- FC apply: outT = (W/sigma) @ pooledT + b_fc
                out_sb0 = cp.tile([D, GPC], F32, tag="outsb0")
                if STAGE >= 5:
                    pfa = tc.alloc_tile_pool(name=f"psum_fca_{_it}", bufs=1,
                                             space="PSUM")
                    op_ = pfa.tile([D, GPC], F32, tag="op")
                    nc.tensor.matmul(out=op_[:], lhsT=wfcT[:], rhs=pooledT[:],
                                     start=True, stop=True)
                    out_sb = cp.tile([D, GPC], F32)
                    nc.scalar.activation(out=out_sb[:], in_=op_[:],
                                         func=mybir.ActivationFunctionType.Identity,
                                         bias=bfc_t[:])
                    nc.sync.dma_start(out=out_d[:], in_=out_sb[:])
                    pfa.release()
                else:
                    nc.vector.memset(out_sb0[:], 0.0)
                    nc.sync.dma_start(out=out_d[:], in_=out_sb0[:])
                hp.release()


    nc.compile()
    return nc


# ----------------------------------------------------------------------------
# Cached executor (compile once per dims signature)
# ----------------------------------------------------------------------------
_CACHE = {}


class _Exec:
    def __init__(self, dims):
        self.dims = dims
        self.nc = _build_program(dims)

    def run(self, in_maps):
        from concourse.bass_utils import run_bass_kernel_spmd
        res = run_bass_kernel_spmd(self.nc, in_maps, list(range(NCORES)))
        return [r["out"] for r in res.results]


def _get_exec(dims):
    key = repr(sorted(dims.items()))
    if key not in _CACHE:
        _CACHE[key] = _Exec(dims)
    return _CACHE[key]


def _make_in_maps(dims, tables, W_gcn, b_gcn, W_fc, b_fc):
    wgcn = np.asarray(W_gcn, dtype=np.float32)
    bgcn = np.asarray(b_gcn, dtype=np.float32).reshape(-1, 1)
    wfc = np.asarray(W_fc, dtype=np.float32)
    bfc = np.asarray(b_fc, dtype=np.float32).reshape(-1, 1)
    gsrc = os.environ.get("GNN_GSRC", "hbm")
    xin = ({"xsb": tables["x_sb"]} if gsrc == "sbuf"
           else {"x": tables["x_pad"]})
    in_maps = []
    for c in range(NCORES):
        in_maps.append({
            **xin,
            "idx": np.ascontiguousarray(tables["idx"][c]),
            "dstl": np.ascontiguousarray(tables["dstl"][c]),
            "sidx": np.ascontiguousarray(tables["sidx"][c]),
            "degp": np.ascontiguousarray(tables["degp"][c]),
            "wgcn": wgcn,
            "bgcn": bgcn,
            "wfc": wfc,
            "bfc": bfc,
        })
    return in_maps


def kernel(x, W_gcn, b_gcn, W_fc, b_fc, edge_index, batch, num_graphs):
    dims, tables = _preprocess(x, edge_index, batch, num_graphs)
    ex = _get_exec(dims)
    in_maps = _make_in_maps(dims, tables, W_gcn, b_gcn, W_fc, b_fc)
    outs = ex.run(in_maps)
    B = dims["B"]
    D = dims["D"]
    result = np.zeros((B, D), dtype=np.float32)
    for c in range(NCORES):
        o = np.asarray(outs[c], dtype=np.float32)  # [D, GPC]
        for gi, g in enumerate(tables["core_graphs"][c]):
            if g >= 0:
                result[g] = o[:, gi]
    return result


# ----------------------------------------------------------------------------
# Reusable jitted runner (for steady-state timing): mirrors
# bass2jax.run_bass_via_pjrt's multi-core path but keeps the jitted callable.
# ----------------------------------------------------------------------------
def _build_jit(nc):
    import jax
    import numpy as _np
    from jax.sharding import Mesh, PartitionSpec
    from jax.experimental.shard_map import shard_map
    from concourse import bass2jax
    from concourse import mybir as _mb

    bass2jax.install_neuronx_cc_hook()
    in_names, out_names, out_avals, zero_outs = [], [], [], []
    partition_name = (nc.partition_id_tensor.name
                      if nc.partition_id_tensor else None)
    for alloc in nc.m.functions[0].allocations:
        if not isinstance(alloc, _mb.MemoryLocationSet):
            continue
        name = alloc.memorylocations[0].name
        if alloc.kind == "ExternalInput":
            if name != partition_name:
                in_names.append(name)
        elif alloc.kind == "ExternalOutput":
            out_names.append(name)
            shape = tuple(alloc.tensor_shape)
            dtype = _mb.dt.np(alloc.dtype)
            out_avals.append(jax.core.ShapedArray(shape, dtype))
            zero_outs.append(_np.zeros(shape, dtype))
    n_params = len(in_names)
    all_in = list(in_names) + list(out_names)
    if partition_name is not None:
        all_in.append(partition_name)

    def _body(*args):
        operands = list(args)
        if partition_name is not None:
            operands.append(bass2jax.partition_id_tensor())
        outs = bass2jax._bass_exec_p.bind(
            *operands,
            out_avals=tuple(out_avals),
            in_names=tuple(all_in),
            out_names=tuple(out_names),
            lowering_input_output_aliases=(),
            sim_require_finite=True,
            sim_require_nnan=True,
            nc=nc,
        )
        return tuple(outs)

    devices = jax.devices()[:NCORES]
    mesh = Mesh(np.asarray(devices), ("core",))
    in_specs = (PartitionSpec("core"),) * (n_params + len(out_names))
    out_specs = (PartitionSpec("core"),) * len(out_names)
    donate = tuple(range(n_params, n_params + len(out_names)))
    fn = jax.jit(
        shard_map(_body, mesh=mesh, in_specs=in_specs, out_specs=out_specs,
                  check_rep=False),
        donate_argnums=donate, keep_unused=True,
    )
    return fn, in_names, out_names, zero_outs


def time_exec(ex, in_maps, reps=24, warmup=4):
    """Median per-iteration device time via back-to-back dispatch."""
    import jax
    import time as _t
    fn, in_names, out_names, zero_outs = _build_jit(ex.nc)
    concat = [np.concatenate([np.asarray(in_maps[c][n]) for c in range(NCORES)],
                             axis=0) for n in in_names]
    dev_in = [jax.device_put(a) for a in concat]
    for a in dev_in:
        a.block_until_ready()

    def zouts():
        return [np.concatenate([z] * NCORES, axis=0) for z in zero_outs]

    def run_n(n):
        outs = None
        t0 = _t.perf_counter()
        for _ in range(n):
            outs = fn(*dev_in, *zouts())
        for o in outs:
            o.block_until_ready()
        return _t.perf_counter() - t0

    run_n(warmup)
    t1 = run_n(reps // 2)
    t2 = run_n(reps)
    per_iter = (t2 - t1) / (reps - reps // 2)
    return per_iter * 1e9


# revision 33
# speedup vs baseline: 2.1034x; 1.8547x over previous
"""GCNHead Trainium2 kernel (8-core SPMD), v2.

Math (matches reference):
  deg = bincount(dst)+1 (self loops);  dinv = deg^-1/2
  agg[n] = sum_{e: dst=n} dinv[src]*dinv[dst] * x[src]   (+ self loop)
  h = agg[n] @ W_gcn + b_gcn
  out = leaky_relu(h, 0.2);  pooled = segment_max(out, batch)
  result = pooled @ (W_fc / sigma_max(W_fc)).T + b_fc

v2 design: the edge norm dinv[src]*dinv[dst] is folded into a VALUED one-hot
(entry = norm instead of 1), so the x~ prescale pass of v1 disappears and the
gather reads raw bf16 x directly (saves a full read+write of x and removes the
serial prefix before gathering can start).  The valued one-hot is built on DVE
(is_equal vs iota, then mult by the norm table; GNN_NDVE batches) with a
gpsimd local_scatter fallback for the rest -- NDVE=14 (all DVE) measured
fastest since Pool is saturated by gather descriptor generation.  The
spectral-norm sigma(W_fc) chain is hoisted out of the repeat loop (it is a
~40-op serial dependency chain that otherwise adds ~100us of semaphore
latency after pooling); bias+leaky_relu is fused into one Activation-engine
Lrelu op.  Device-verified limits: dma_gather calls are capped at 1024
descriptors; SBUF-source dma_gather is not supported by this runtime.

Host does integer-only preprocessing (sharding, bucketing, index tables,
degree products via bincount); all float math runs on device.
"""
import sys

sys.path.insert(0, "/opt/trn_rl_repo")

import math
import os
import numpy as np
import ml_dtypes

import concourse.bass as bass
import concourse.mybir as mybir
import concourse.tile as tile
from concourse import bacc
from concourse.masks import make_identity

BF16 = mybir.dt.bfloat16
F32 = mybir.dt.float32
I16 = mybir.dt.int16

NCORES = 8
SENT = 20000.0  # dst_local sentinel (never matches iota 0..63)
BB = 8          # tiles per bank-batch
CH = 30         # blocks per local_scatter call (30*64=1920 < 2048 elems)


# ----------------------------------------------------------------------------
# Host preprocessing (integers only)
# ----------------------------------------------------------------------------
def _preprocess(x, edge_index, batch, num_graphs):
    N, D = x.shape
    B = int(num_graphs)
    src = np.asarray(edge_index[0], dtype=np.int64)
    dst = np.asarray(edge_index[1], dtype=np.int64)
    batch = np.asarray(batch, dtype=np.int64)

    deg = np.bincount(dst, minlength=N).astype(np.int64) + 1  # + self loop

    # graph -> node range (batch sorted)
    counts_g = np.bincount(batch, minlength=B)
    starts_g = np.concatenate([[0], np.cumsum(counts_g)])

    GPC = math.ceil(B / NCORES)  # graphs per core
    # balance graphs across cores by total edge weight (snake over sorted)
    gw = np.add.reduceat(deg, starts_g[:-1]) if N else counts_g
    gw = np.where(counts_g > 0, gw, 0)
    order = np.argsort(-gw, kind="stable")
    core_graphs = [[] for _ in range(NCORES)]
    loads = np.zeros(NCORES)
    for g in order:
        c = int(np.argmin([loads[i] + (1e18 if len(core_graphs[i]) >= GPC else 0)
                           for i in range(NCORES)]))
        core_graphs[c].append(int(g))
        loads[c] += gw[g]
    for c in range(NCORES):
        core_graphs[c] += [-1] * (GPC - len(core_graphs[c]))

    GCAP = 64 * max(1, math.ceil(counts_g.max() / 64))
    TPG = GCAP // 64          # tiles per graph
    TT = GPC * TPG            # tiles per core
    S = TT * 64               # slots per core

    # --- slot assignment: per graph, balance node degree across TPG bins ---
    node_slot = np.full(N, -1, dtype=np.int64)   # slot within its core
    node_core = np.full(N, -1, dtype=np.int64)
    for c in range(NCORES):
        for gi, g in enumerate(core_graphs[c]):
            if g < 0:
                continue
            nodes = np.arange(starts_g[g], starts_g[g + 1])
            if len(nodes) == 0:
                continue
            nd = deg[nodes]
            ordn = np.argsort(-nd, kind="stable")
            binload = np.zeros(TPG, dtype=np.int64)
            binfill = np.zeros(TPG, dtype=np.int64)
            for i in ordn:
                masked = np.where(binfill < 64, binload, np.iinfo(np.int64).max)
                b = int(np.argmin(masked))
                slot = gi * GCAP + b * 64 + binfill[b]
                node_slot[nodes[i]] = slot
                node_core[nodes[i]] = c
                binfill[b] += 1
                binload[b] += nd[i]

    # --- edges (incl self loops) bucketed per (core, tile, parity) ---
    loop = np.arange(N, dtype=np.int64)
    esrc = np.concatenate([src, loop])
    edst = np.concatenate([dst, loop])
    ecore = node_core[edst]
    eslot = node_slot[edst]
    etile = eslot >> 6
    edl = (eslot & 63).astype(np.int64)
    epar = (esrc & 1).astype(np.int64)

    # per-core lists of (tile, parity) buckets; order edges by key
    counts = np.zeros((NCORES, TT, 2), dtype=np.int64)
    per_core_order = []
    for c in range(NCORES):
        sel = np.where(ecore == c)[0]
        k = etile[sel] * 2 + epar[sel]
        o = np.argsort(k, kind="stable")
        sel = sel[o]
        per_core_order.append(sel)
        cnt = np.bincount(k[o], minlength=TT * 2)
        counts[c] = cnt.reshape(TT, 2)

    # class capacity = max over cores, rounded to 128
    cap = ((counts.max(axis=0) + 127) // 128) * 128      # [TT, 2]
    blocks = cap // 128                                   # [TT, 2]
    TOTBLK = int(blocks.sum())
    TOTPOS = TOTBLK * 128

    # global block layout: tiles ascending, class even then odd
    class_off = np.zeros((TT, 2), dtype=np.int64)        # position offsets
    pos = 0
    for t in range(TT):
        for q in range(2):
            class_off[t, q] = pos
            pos += cap[t, q]

    # per-tile first block; bank-batch block spans (even-rounded for scatter)
    tile_blk0 = np.zeros(TT + 1, dtype=np.int64)
    for t in range(TT):
        tile_blk0[t + 1] = tile_blk0[t] + blocks[t, 0] + blocks[t, 1]
    nbatch = math.ceil(TT / BB)
    batch_blk0 = np.array([tile_blk0[min(b * BB, TT)] for b in range(nbatch + 1)])
    batch_nblk = np.diff(batch_blk0)
    batch_nblk_e = (batch_nblk + 1) // 2 * 2              # even-rounded
    ebl0 = np.concatenate([[0], np.cumsum(batch_nblk_e)])
    TOTBLKE = int(ebl0[-1])

    # tables
    idx_tab = np.zeros((NCORES, TOTPOS), dtype=np.int64)
    dstl_tab = np.full((NCORES, 128, TOTBLKE), SENT, dtype=np.float32)
    sidx_tab = np.full((NCORES, 128, TOTBLKE), -1, dtype=np.int16)
    degp_tab = np.ones((NCORES, 128, TOTBLKE), dtype=np.float32)
    blk_batch = np.searchsorted(batch_blk0[1:], np.arange(TOTBLK), side="right")
    blk_ecol = ebl0[blk_batch] + (np.arange(TOTBLK) - batch_blk0[blk_batch])
    blk_local = np.arange(TOTBLK) - batch_blk0[blk_batch]
    for c in range(NCORES):
        sel = per_core_order[c]
        k = etile[sel] * 2 + epar[sel]
        # position within class = running index per class
        cstart = np.concatenate([[0], np.cumsum(np.bincount(k, minlength=TT * 2))])
        within = np.arange(len(sel)) - cstart[k]
        gpos = class_off.reshape(-1)[k] + within
        idx_tab[c, gpos] = esrc[sel] >> 1
        blk = gpos >> 7
        lane = gpos & 127
        ecol = blk_ecol[blk]
        dstl_tab[c, lane, ecol] = edl[sel]
        sidx_tab[c, lane, ecol] = (blk_local[blk] % CH) * 64 + edl[sel]
        degp_tab[c, lane, ecol] = deg[esrc[sel]] * deg[edst[sel]]

    # idx table SBUF layout [128, TOTPOS/16]: flat i -> [i%16 (+16r), i//16]
    idx16 = idx_tab.astype(np.int16).reshape(NCORES, TOTPOS // 16, 16)
    idx16 = np.ascontiguousarray(idx16.transpose(0, 2, 1))           # [NC,16,P/16]
    idx128 = np.tile(idx16, (1, 8, 1))                               # [NC,128,...]
    dstl128 = dstl_tab.astype(ml_dtypes.bfloat16)

    NP = ((N + 255) // 256) * 256          # pad to even multiple of 128
    x_pad = np.zeros((NP, D), dtype=ml_dtypes.bfloat16)
    x_pad[:N] = np.asarray(x, dtype=np.float32).astype(ml_dtypes.bfloat16)
    # SBUF-resident pair layout: pair p -> partition p%128, rank p//128
    NPAIR = NP // 2
    RNK = NPAIR // 128
    x_sb = np.ascontiguousarray(
        x_pad.reshape(RNK, 128, 2 * D).transpose(1, 0, 2)
    ).reshape(128, RNK * 2 * D)

    dims = dict(N=N, D=D, B=B, GPC=GPC, GCAP=GCAP, TPG=TPG, TT=TT, S=S, NP=NP,
                TOTBLK=TOTBLK, TOTPOS=TOTPOS, TOTBLKE=TOTBLKE,
                blocks=tuple(map(tuple, blocks)),
                kmax=tuple(map(tuple, counts.max(axis=0))),
                batch_nblk_e=tuple(batch_nblk_e), ebl0=tuple(ebl0))
    tables = dict(idx=idx128, dstl=dstl128, sidx=sidx_tab, degp=degp_tab,
                  x_pad=x_pad, x_sb=x_sb, core_graphs=core_graphs)
    return dims, tables


# ----------------------------------------------------------------------------
# Device program
# ----------------------------------------------------------------------------
def _build_program(dims):
    D = dims["D"]
    TT, TPG, GPC, GCAP = dims["TT"], dims["TPG"], dims["GPC"], dims["GCAP"]
    NP, TOTBLK, TOTPOS = dims["NP"], dims["TOTBLK"], dims["TOTPOS"]
    TOTBLKE = dims["TOTBLKE"]
    blocks = dims["blocks"]
    kmax = dims["kmax"]
    batch_nblk_e = dims["batch_nblk_e"]
    ebl0 = dims["ebl0"]
    S = dims["S"]

    SCRATCH = int(os.environ.get("GNN_SCRATCH", "16384"))
    GSTEP = int(os.environ.get("GNN_GSTEP", "8"))         # blocks per gather
    # NOTE: the device rejects gather calls over 1024 descriptors (GSTEP>8).
    NDVE = int(os.environ.get("GNN_NDVE", "14"))          # batches on DVE path
    NQ = int(os.environ.get("GNN_NQ", "4"))
    LRELU = int(os.environ.get("GNN_LRELU", "1"))
    GSRC = os.environ.get("GNN_GSRC", "hbm")              # hbm | sbuf
    SPKT = bool(int(os.environ.get("GNN_SPKT", "1")))

    nc = bacc.Bacc("TRN2", target_bir_lowering=False, debug=False,
                   num_swdge_queues=NQ, dynamic_dma_scratch_size=SCRATCH)
    if GSRC == "sbuf":
        RNK = (NP // 2) // 128
        xsb_d = nc.dram_tensor("xsb", [128, RNK * 2 * D], BF16,
                               kind="ExternalInput")
    else:
        x_d = nc.dram_tensor("x", [NP, D], BF16, kind="ExternalInput")
    idx_d = nc.dram_tensor("idx", [128, TOTPOS // 16], I16, kind="ExternalInput")
    dstl_d = nc.dram_tensor("dstl", [128, TOTBLKE], BF16, kind="ExternalInput")
    sidx_d = nc.dram_tensor("sidx", [128, TOTBLKE], I16, kind="ExternalInput")
    degp_d = nc.dram_tensor("degp", [128, TOTBLKE], F32, kind="ExternalInput")
    wgcn_d = nc.dram_tensor("wgcn", [D, D], F32, kind="ExternalInput")
    bgcn_d = nc.dram_tensor("bgcn", [D, 1], F32, kind="ExternalInput")
    wfc_d = nc.dram_tensor("wfc", [D, D], F32, kind="ExternalInput")
    bfc_d = nc.dram_tensor("bfc", [D, 1], F32, kind="ExternalInput")
    out_d = nc.dram_tensor("out", [D, GPC], F32, kind="ExternalOutput")

    # per-tile block ranges + max blocks in any bank-batch
    tile_blk0 = []
    pos = 0
    for t in range(TT):
        tile_blk0.append((pos, blocks[t][0], blocks[t][1]))
        pos += blocks[t][0] + blocks[t][1]
    tile_blk0.append((pos, 0, 0))
    GMAX = 0
    GMAXE = max(batch_nblk_e)
    for b0 in range(0, TT, BB):
        nb = min(BB, TT - b0)
        GMAX = max(GMAX, tile_blk0[b0 + nb][0] - tile_blk0[b0][0])

    with tile.TileContext(nc) as tc:
        with (
            tc.tile_pool(name="consts", bufs=1) as cp,
        ):
            # ---------------- constants / tables ----------------
            idx_t = cp.tile([128, TOTPOS // 16], I16)
            nc.sync.dma_start(out=idx_t[:], in_=idx_d[:])
            dstl_t = cp.tile([128, TOTBLKE], BF16)
            nc.sync.dma_start(out=dstl_t[:], in_=dstl_d[:])
            sidx_t = cp.tile([128, TOTBLKE], I16)
            nc.sync.dma_start(out=sidx_t[:], in_=sidx_d[:])
            degp_t = cp.tile([128, TOTBLKE], F32)
            nc.sync.dma_start(out=degp_t[:], in_=degp_d[:])
            wgcn_t = cp.tile([D, D], F32)
            nc.sync.dma_start(out=wgcn_t[:], in_=wgcn_d[:])
            bgcn_t = cp.tile([D, 1], F32)
            nc.sync.dma_start(out=bgcn_t[:], in_=bgcn_d[:])
            wfc_t = cp.tile([D, D], F32)
            nc.sync.dma_start(out=wfc_t[:], in_=wfc_d[:])
            bfc_t = cp.tile([D, 1], F32)
            nc.sync.dma_start(out=bfc_t[:], in_=bfc_d[:])

            iota_t = cp.tile([128, 64], BF16)
            nc.gpsimd.iota(iota_t[:], pattern=[[1, 64]], base=0,
                           channel_multiplier=0,
                           allow_small_or_imprecise_dtypes=True)
            ident_t = cp.tile([128, 128], F32)
            make_identity(nc, ident_t[:])
            if GSRC == "sbuf":
                # I(64) stacked twice so slices at base partition 0 and 64 both
                # see an identity (transpose operands must share base partition)
                identb_t = cp.tile([128, 64], BF16)
                make_identity(nc, identb_t[0:64, :])
                make_identity(nc, identb_t[64:128, :])

            # normv = (deg_src*deg_dst)^-1/2 in bf16
            rec_t = cp.tile([128, TOTBLKE], F32)
            nc.vector.reciprocal(rec_t[:], degp_t[:])
            nc.scalar.activation(rec_t[:], rec_t[:],
                                 mybir.ActivationFunctionType.Sqrt)
            normv_t = cp.tile([128, TOTBLKE], BF16)
            nc.vector.tensor_copy(out=normv_t[:], in_=rec_t[:])

            if GSRC == "sbuf":
                xsb_t = cp.tile([128, RNK * 2 * D], BF16)
                nc.sync.dma_start(out=xsb_t[:], in_=xsb_d[:])
            else:
                x_pairs = x_d[:].rearrange("(v two) d -> v (two d)", two=2)

            REPEAT = int(os.environ.get("GNN_REPEAT", "1"))
            STAGE = int(os.environ.get("GNN_STAGE", "9"))
            _QRR = [0]

            # ---- spectral norm sigma(W_fc) -> wfcT (iteration-invariant,
            # overlaps the gather phase instead of serializing after pooling)
            wfcT = cp.tile([D, D], F32)
            if STAGE >= 5:
                pf = tc.alloc_tile_pool(name="psum_fc", bufs=1, space="PSUM")
                mp = pf.tile([D, D], F32, tag="mp")
                nc.tensor.matmul(out=mp[:], lhsT=wfc_t[:], rhs=wfc_t[:],
                                 start=True, stop=True)
                m1_sb = cp.tile([D, D], F32, tag="m1sb")
                nc.scalar.copy(out=m1_sb[:], in_=mp[:])
                cur = m1_sb
                for _ in range(6):  # M^64
                    mp2 = pf.tile([D, D], F32, tag="mp")
                    nc.tensor.matmul(out=mp2[:], lhsT=cur[:], rhs=cur[:],
                                     start=True, stop=True)
                    nxt = cp.tile([D, D], F32, tag=f"m{_}")
                    nc.scalar.copy(out=nxt[:], in_=mp2[:])
                    cur = nxt
                ones_c = cp.tile([D, 1], F32)
                nc.vector.memset(ones_c[:], 1.0)
                ones_r = cp.tile([1, D], F32)
                nc.vector.memset(ones_r[:], 1.0)
                vp = pf.tile([D, 1], F32, tag="vp")
                nc.tensor.matmul(out=vp[:], lhsT=cur[:], rhs=ones_c[:],
                                 start=True, stop=True)
                v_sb = cp.tile([D, 1], F32)
                nc.scalar.copy(out=v_sb[:], in_=vp[:])
                wp = pf.tile([D, 1], F32, tag="vp")
                nc.tensor.matmul(out=wp[:], lhsT=m1_sb[:], rhs=v_sb[:],
                                 start=True, stop=True)
                w_sb = cp.tile([D, 1], F32)
                nc.scalar.copy(out=w_sb[:], in_=wp[:])
                nump = pf.tile([1, 1], F32, tag="sc")
                nc.tensor.matmul(out=nump[:], lhsT=v_sb[:], rhs=w_sb[:],
                                 start=True, stop=True)
                denp = pf.tile([1, 1], F32, tag="sc")
                nc.tensor.matmul(out=denp[:], lhsT=v_sb[:], rhs=v_sb[:],
                                 start=True, stop=True)
                num_sb = cp.tile([1, 1], F32, tag="num")
                den_sb = cp.tile([1, 1], F32, tag="den")
                nc.vector.tensor_copy(out=num_sb[:], in_=nump[:])
                nc.vector.tensor_copy(out=den_sb[:], in_=denp[:])
                rinv = cp.tile([1, 1], F32, tag="rinv")
                nc.vector.reciprocal(rinv[:], num_sb[:])
                nc.vector.tensor_tensor(out=rinv[:], in0=rinv[:], in1=den_sb[:],
                                        op=mybir.AluOpType.mult)
                nc.scalar.activation(rinv[:], rinv[:],
                                     mybir.ActivationFunctionType.Sqrt)
                sp = pf.tile([D, 1], F32, tag="vp")
                nc.tensor.matmul(out=sp[:], lhsT=ones_r[:], rhs=rinv[:],
                                 start=True, stop=True)
                s_col = cp.tile([D, 1], F32)
                nc.scalar.copy(out=s_col[:], in_=sp[:])

                # W_fc^T, scaled by 1/sigma
                wtp = pf.tile([D, D], F32, tag="mp")
                nc.tensor.transpose(out=wtp[:], in_=wfc_t[:],
                                    identity=ident_t[:D, :D])
                nc.vector.tensor_scalar_mul(wfcT[:], wtp[:], s_col[:])
                pf.release()
            for _it in range(REPEAT):
                hp = tc.alloc_tile_pool(name=f"h_{_it}", bufs=1)
                hT = hp.tile([64, S], F32)

                gp = tc.alloc_tile_pool(name=f"gath_{_it}", bufs=2)
                ohp = tc.alloc_tile_pool(name=f"oh_{_it}", bufs=2)
                tp = tc.alloc_tile_pool(name=f"tail_{_it}", bufs=2)
                pa = tc.alloc_tile_pool(name=f"psum_acc_{_it}", bufs=2, space="PSUM")
                pm = tc.alloc_tile_pool(name=f"psum_misc_{_it}", bufs=2, space="PSUM")
                tpp = (tc.alloc_tile_pool(name=f"psum_tr_{_it}", bufs=2,
                                          space="PSUM")
                       if GSRC == "sbuf" else None)
                for bi, b0 in enumerate(range(0, TT, BB) if STAGE >= 2 else []):
                    nb = min(BB, TT - b0)
                    blk0 = tile_blk0[b0][0]
                    blk1 = tile_blk0[b0 + nb][0] if b0 + nb < TT else TOTBLK
                    nblk = blk1 - blk0
                    nblk_e = batch_nblk_e[bi]
                    e0 = ebl0[bi]
                    if nblk > 0:
                        # class (parity) of each block in this batch
                        blkq = []
                        for t in range(b0, b0 + nb):
                            _, be_, bo_ = tile_blk0[t]
                            blkq += [0] * be_ + [1] * bo_
                        if GSRC == "sbuf":
                            gT = gp.tile([128, GMAX * 128], BF16, tag="gT")
                            for g0 in range(0, nblk, GSTEP):
                                gl = min(GSTEP, nblk - g0)
                                nc.gpsimd.dma_gather(
                                    out_ap=gT[:, g0 * 128:(g0 + gl) * 128]
                                    .rearrange("p (one n) -> p one n", one=1),
                                    in_ap=xsb_t[:],
                                    idxs_ap=idx_t[
                                        :, (blk0 + g0) * 8:(blk0 + g0 + gl) * 8],
                                    num_idxs=gl * 128,
                                    num_idxs_reg=gl * 128,
                                    elem_size=128,
                                    transpose=True,
                                    sbuf_tokens_per_rank=128,
                                    sbuf_free_dim_per_rank=2 * D * 2,
                                    queue_num=_QRR[0],
                                )
                                _QRR[0] = (_QRR[0] + 1) % NQ
                            if STAGE < 3:
                                continue
                            # PE-transpose to edge-major, class half selected
                            gat = gp.tile([128, GMAX, 64], BF16, tag="gat")
                            for g0 in range(0, nblk, 8):
                                gl = min(8, nblk - g0)
                                trg = tpp.tile([128, 8, 64], BF16, tag="trg")
                                for j in range(gl):
                                    lb = g0 + j
                                    q = blkq[lb]
                                    nc.tensor.transpose(
                                        out=trg[:, j, :],
                                        in_=gT[64 * q:64 * q + 64,
                                               lb * 128:(lb + 1) * 128],
                                        identity=identb_t[
                                            64 * q:64 * q + 64, :],
                                    )
                                nc.scalar.copy(out=gat[:, g0:g0 + gl, :],
                                               in_=trg[:, :gl, :])
                        else:
                            gat = gp.tile([128, GMAX, 128], BF16, tag="gat")
                            for g0 in range(0, nblk, GSTEP):
                                gl = min(GSTEP, nblk - g0)
                                nc.gpsimd.dma_gather(
                                    out_ap=gat[:, g0:g0 + gl, :],
                                    in_ap=x_pairs,
                                    idxs_ap=idx_t[
                                        :, (blk0 + g0) * 8:(blk0 + g0 + gl) * 8],
                                    num_idxs=gl * 128,
                                    num_idxs_reg=gl * 128,
                                    elem_size=128,
                                    single_packet=SPKT,
                                    queue_num=_QRR[0],
                                )
                                _QRR[0] = (_QRR[0] + 1) % NQ
                            if STAGE < 3:
                                continue
                        # valued one-hot [128, nblk_e, 64]
                        ohv = ohp.tile([128, GMAXE, 64], BF16, tag="ohv")
                        if bi < NDVE:
                            nc.vector.tensor_tensor(
                                out=ohv[:, :nblk_e, :],
                                in0=dstl_t[:, e0:e0 + nblk_e].to_broadcast(
                                    [128, nblk_e, 64]),
                                in1=iota_t[:, :64].rearrange(
                                    "p (j m) -> p j m", j=1).to_broadcast(
                                    [128, nblk_e, 64]),
                                op=mybir.AluOpType.is_equal,
                            )
                            nc.vector.tensor_tensor(
                                out=ohv[:, :nblk_e, :],
                                in0=ohv[:, :nblk_e, :],
                                in1=normv_t[:, e0:e0 + nblk_e].to_broadcast(
                                    [128, nblk_e, 64]),
                                op=mybir.AluOpType.mult,
                            )
                        else:
                            for c0 in range(0, nblk_e, CH):
                                ch = min(CH, nblk_e - c0)
                                nc.gpsimd.local_scatter(
                                    out_ap=ohv[:, c0:c0 + ch, :].rearrange(
                                        "p a b -> p (a b)"),
                                    data_ap=normv_t[:, e0 + c0:e0 + c0 + ch],
                                    idxs_ap=sidx_t[:, e0 + c0:e0 + c0 + ch],
                                    channels=128,
                                    num_elems=ch * 64,
                                    num_idxs=ch,
                                )
                    acc = pa.tile([64, BB, 64], F32, tag="acc")
                    if nblk == 0:
                        nc.vector.memset(acc[:], 0.0)
                    for ti in range(nb):
                        t = b0 + ti
                        base, be, bo = tile_blk0[t]
                        nblks_t = be + bo
                        first = True
                        for j in range(nblks_t):
                            q = 0 if j < be else 1
                            jj = j if j < be else j - be
                            k = 128
                            if jj == (be if q == 0 else bo) - 1:
                                k = kmax[t][q] - 128 * jj
                                k = 128 if k <= 0 else k
                            lb = base - blk0 + j
                            xop = (gat[:k, lb, :] if GSRC == "sbuf"
                                   else gat[:k, lb, 64 * q:64 * q + 64])
                            # lhsT = gathered x -> acc comes out [dim, slot],
                            # already transposed for the W_gcn transform
                            nc.tensor.matmul(
                                out=acc[:, ti, :],
                                lhsT=xop,
                                rhs=ohv[:k, lb, :],
                                start=first,
                                stop=(j == nblks_t - 1),
                            )
                            first = False
                        if nblks_t == 0 and nblk > 0:
                            nc.vector.memset(acc[:, ti, :], 0.0)

                    if STAGE < 4:
                        continue
                    # tail per batch: acc is already [dim, slot]; one copy out
                    # of PSUM, then the W_gcn transform
                    aggT = tp.tile([64, BB, 64], F32, tag="aggT")
                    nc.scalar.copy(out=aggT[:, :nb, :], in_=acc[:, :nb, :])
                    hps = pm.tile([64, BB * 64], F32, tag="hps")
                    nc.tensor.matmul(
                        out=hps[:, :nb * 64],
                        lhsT=wgcn_t[:],
                        rhs=aggT[:, :nb, :].rearrange("p a b -> p (a b)"),
                        start=True, stop=True,
                    )
                    if LRELU:
                        # fused bias + leaky_relu(0.2) on the Activation engine
                        nc.scalar.activation(
                            out=hT[:, b0 * 64:(b0 + nb) * 64],
                            in_=hps[:, :nb * 64],
                            func=mybir.ActivationFunctionType.Lrelu,
                            bias=bgcn_t[:], alpha=0.2,
                        )
                    else:
                        nc.scalar.activation(
                            out=hT[:, b0 * 64:(b0 + nb) * 64], in_=hps[:, :nb * 64],
                            func=mybir.ActivationFunctionType.Identity,
                            bias=bgcn_t[:],
                        )
                        # leaky relu on this batch's columns
                        lk = tp.tile([64, BB * 64], F32, tag="lk")
                        nc.vector.tensor_scalar_mul(
                            lk[:, :nb * 64], hT[:, b0 * 64:(b0 + nb) * 64], 0.2)
                        nc.vector.tensor_tensor(
                            out=hT[:, b0 * 64:(b0 + nb) * 64],
                            in0=hT[:, b0 * 64:(b0 + nb) * 64],
                            in1=lk[:, :nb * 64], op=mybir.AluOpType.max)

                for _pool in ((tpp,) if tpp else ()) + (pm, pa, tp, ohp, gp):
                    _pool.release()

                # ---------------- pooling ----------------
                pooledT = hp.tile([64, GPC], F32)
                for g in (range(GPC) if STAGE >= 5 else []):
                    nc.vector.tensor_reduce(
                        out=pooledT[:, g:g + 1],
                        in_=hT[:, g * GCAP:(g + 1) * GCAP],
                        axis=mybir.AxisListType.X,
                        op=mybir.AluOpType.max,
                    )

                # ---------------- FC apply: outT = (W/sigma) @ pooledT + b_fc
                out_sb0 = cp.tile([D, GPC], F32, tag="outsb0")
                if STAGE >= 5:
                    pfa = tc.alloc_tile_pool(name=f"psum_fca_{_it}", bufs=1,
                                             space="PSUM")
                    op_ = pfa.tile([D, GPC], F32, tag="op")
                    nc.tensor.matmul(out=op_[:], lhsT=wfcT[:], rhs=pooledT[:],
                                     start=True, stop=True)
                    out_sb = cp.tile([D, GPC], F32)
                    nc.scalar.activation(out=out_sb[:], in_=op_[:],
                                         func=mybir.ActivationFunctionType.Identity,
                                         bias=bfc_t[:])
                    nc.sync.dma_start(out=out_d[:], in_=out_sb[:])
                    pfa.release()
                else:
                    nc.vector.memset(out_sb0[:], 0.0)
                    nc.sync.dma_start(out=out_d[:], in_=out_sb0[:])
                hp.release()


    nc.compile()
    return nc


# ----------------------------------------------------------------------------
# Cached executor (compile once per dims signature)
# ----------------------------------------------------------------------------
_CACHE = {}


class _Exec:
    def __init__(self, dims):
        self.dims = dims
        self.nc = _build_program(dims)

    def run(self, in_maps):
        from concourse.bass_utils import run_bass_kernel_spmd
        res = run_bass_kernel_spmd(self.nc, in_maps, list(range(NCORES)))
        return [r["out"] for r in res.results]


def _get_exec(dims):
    key = repr(sorted(dims.items()))
    if key not in _CACHE:
        _CACHE[key] = _Exec(dims)
    return _CACHE[key]


def _make_in_maps(dims, tables, W_gcn, b_gcn, W_fc, b_fc):
    wgcn = np.asarray(W_gcn, dtype=np.float32)
    bgcn = np.asarray(b_gcn, dtype=np.float32).reshape(-1, 1)
    wfc = np.asarray(W_fc, dtype=np.float32)
    bfc = np.asarray(b_fc, dtype=np.float32).reshape(-1, 1)
    gsrc = os.environ.get("GNN_GSRC", "hbm")
    xin = ({"xsb": tables["x_sb"]} if gsrc == "sbuf"
           else {"x": tables["x_pad"]})
    in_maps = []
    for c in range(NCORES):
        in_maps.append({
            **xin,
            "idx": np.ascontiguousarray(tables["idx"][c]),
            "dstl": np.ascontiguousarray(tables["dstl"][c]),
            "sidx": np.ascontiguousarray(tables["sidx"][c]),
            "degp": np.ascontiguousarray(tables["degp"][c]),
            "wgcn": wgcn,
            "bgcn": bgcn,
            "wfc": wfc,
            "bfc": bfc,
        })
    return in_maps


def kernel(x, W_gcn, b_gcn, W_fc, b_fc, edge_index, batch, num_graphs):
    dims, tables = _preprocess(x, edge_index, batch, num_graphs)
    ex = _get_exec(dims)
    in_maps = _make_in_maps(dims, tables, W_gcn, b_gcn, W_fc, b_fc)
    outs = ex.run(in_maps)
    B = dims["B"]
    D = dims["D"]
    result = np.zeros((B, D), dtype=np.float32)
    for c in range(NCORES):
        o = np.asarray(outs[c], dtype=np.float32)  # [D, GPC]
        for gi, g in enumerate(tables["core_graphs"][c]):
            if g >= 0:
                result[g] = o[:, gi]
    return result


# ----------------------------------------------------------------------------
# Reusable jitted runner (for steady-state timing): mirrors
# bass2jax.run_bass_via_pjrt's multi-core path but keeps the jitted callable.
# ----------------------------------------------------------------------------
def _build_jit(nc):
    import jax
    import numpy as _np
    from jax.sharding import Mesh, PartitionSpec
    from jax.experimental.shard_map import shard_map
    from concourse import bass2jax
    from concourse import mybir as _mb

    bass2jax.install_neuronx_cc_hook()
    in_names, out_names, out_avals, zero_outs = [], [], [], []
    partition_name = (nc.partition_id_tensor.name
                      if nc.partition_id_tensor else None)
    for alloc in nc.m.functions[0].allocations:
        if not isinstance(alloc, _mb.MemoryLocationSet):
            continue
        name = alloc.memorylocations[0].name
        if alloc.kind == "ExternalInput":
            if name != partition_name:
                in_names.append(name)
        elif alloc.kind == "ExternalOutput":
            out_names.append(name)
            shape = tuple(alloc.tensor_shape)
            dtype = _mb.dt.np(alloc.dtype)
            out_avals.append(jax.core.ShapedArray(shape, dtype))
            zero_outs.append(_np.zeros(shape, dtype))
    n_params = len(in_names)
    all_in = list(in_names) + list(out_names)
    if partition_name is not None:
        all_in.append(partition_name)

    def _body(*args):
        operands = list(args)
        if partition_name is not None:
            operands.append(bass2jax.partition_id_tensor())
        outs = bass2jax._bass_exec_p.bind(
            *operands,
            out_avals=tuple(out_avals),
            in_names=tuple(all_in),
            out_names=tuple(out_names),
            lowering_input_output_aliases=(),
            sim_require_finite=True,
            sim_require_nnan=True,
            nc=nc,
        )
        return tuple(outs)

    devices = jax.devices()[:NCORES]
    mesh = Mesh(np.asarray(devices), ("core",))
    in_specs = (PartitionSpec("core"),) * (n_params + len(out_names))
    out_specs = (PartitionSpec("core"),) * len(out_names)
    donate = tuple(range(n_params, n_params + len(out_names)))
    fn = jax.jit(
        shard_map(_body, mesh=mesh, in_specs=in_specs, out_specs=out_specs,
                  check_rep=False),
        donate_argnums=donate, keep_unused=True,
    )
    return fn, in_names, out_names, zero_outs


def time_exec(ex, in_maps, reps=24, warmup=4):
    """Median per-iteration device time via back-to-back dispatch."""
    import jax
    import time as _t
    fn, in_names, out_names, zero_outs = _build_jit(ex.nc)
    concat = [np.concatenate([np.asarray(in_maps[c][n]) for c in range(NCORES)],
                             axis=0) for n in in_names]
    dev_in = [jax.device_put(a) for a in concat]
    for a in dev_in:
        a.block_until_ready()

    def zouts():
        return [np.concatenate([z] * NCORES, axis=0) for z in zero_outs]

    def run_n(n):
        outs = None
        t0 = _t.perf_counter()
        for _ in range(n):
            outs = fn(*dev_in, *zouts())
        for o in outs:
            o.block_until_ready()
        return _t.perf_counter() - t0

    run_n(warmup)
    t1 = run_n(reps // 2)
    t2 = run_n(reps)
    per_iter = (t2 - t1) / (reps - reps // 2)
    return per_iter * 1e9
